# revision 2
# baseline (speedup 1.0000x reference)
"""Multi-head self-attention with low-rank bilinear scores (LSR) on 8 TRN2 cores.

Sharding: core i handles batch b = i//4 and 4 heads (i%4)*4..+4.
Per core, everything runs in the "transposed" orientation:
  qkv^T = W^T @ x^T   (contract over D, x pre-transposed on host)
  ql^T / kl^T via block-diagonal packed LSR weights (2 heads per matmul)
  scores^T [s, t] = kl^T.T-slices @ ql^T  (K = RANK = 32, per-head 32-row strips)
  P = exp(scores^T) * causal_mask;  PV uses V with an appended ones column so
  the softmax denominator falls out of the same matmul (row 64 of y_psum).
  y^T gets normalized by 1/den (broadcast via a K=1 ones matmul), then the
  W_o row-slice projection produces a per-core partial [T, D] output.
Host: per-batch sum over the 4 cores sharing that batch (W_o is row-sharded),
then stack the two batches.
"""

import numpy as np
from contextlib import ExitStack

import concourse.tile as tile
from concourse import bacc, mybir
from concourse.bass_utils import run_bass_kernel_spmd

B, T, D = 2, 2048, 1024
H, RANK = 16, 32
DH = D // H  # 64
N_CORES = 8
HPC = 4  # heads per core
CPB = N_CORES // B  # cores per batch
W = 512  # t-window (= one fp32 PSUM bank)
KC = D // 128  # k-chunks over D
F32 = mybir.dt.float32

_NC_CACHE = {}


def _build_nc(t_len=T, mm_dtype=F32):
    """Build + compile the single-core SPMD program (same NEFF on all cores)."""
    NW = t_len // W  # windows
    SBW = W // 128  # s-chunks per window
    NSB = t_len // 128  # total s-chunks

    def mm(ap):
        return ap if mm_dtype == F32 else ap.bitcast(mm_dtype)

    nc = bacc.Bacc("TRN2", target_bir_lowering=False, debug=False,
                   num_devices=N_CORES)

    xt_d = nc.dram_tensor("xt", [D, t_len], F32, kind="ExternalInput")
    wqk_d = nc.dram_tensor("wqk", [128, KC, 2 * HPC * DH], F32, kind="ExternalInput")
    wv_d = nc.dram_tensor("wv", [128, KC, HPC * DH], F32, kind="ExternalInput")
    wql_d = nc.dram_tensor("wql", [128, HPC // 2, 2 * RANK], F32, kind="ExternalInput")
    wkl_d = nc.dram_tensor("wkl", [128, HPC // 2, 2 * RANK], F32, kind="ExternalInput")
    wo_d = nc.dram_tensor("wo", [128, 2, D], F32, kind="ExternalInput")
    mask_d = nc.dram_tensor("masks", [128, SBW, W], F32, kind="ExternalInput")
    out_d = nc.dram_tensor("out", [t_len, D], F32, kind="ExternalOutput")

    with tile.TileContext(nc) as tc, ExitStack() as ctx:
        const = ctx.enter_context(tc.tile_pool(name="const", bufs=1))
        xt_pool = ctx.enter_context(tc.tile_pool(name="xtp", bufs=2))
        qkt_pool = ctx.enter_context(tc.tile_pool(name="qktp", bufs=2))
        klt_pool = ctx.enter_context(tc.tile_pool(name="kltp", bufs=NW))
        qlt_pool = ctx.enter_context(tc.tile_pool(name="qltp", bufs=2))
        v_pool = ctx.enter_context(tc.tile_pool(name="vp", bufs=NSB))
        p_pool = ctx.enter_context(tc.tile_pool(name="pp", bufs=3))
        y_pool = ctx.enter_context(tc.tile_pool(name="yp", bufs=2))
        dr_pool = ctx.enter_context(tc.tile_pool(name="drp", bufs=2))
        stage_pool = ctx.enter_context(tc.tile_pool(name="stgp", bufs=2))
        acc = ctx.enter_context(tc.tile_pool(name="accp", bufs=4, space="PSUM"))
        stp = ctx.enter_context(tc.tile_pool(name="stpp", bufs=4, space="PSUM"))

        wqk_sb = const.tile([128, KC, 2 * HPC * DH], F32)
        nc.sync.dma_start(wqk_sb[:], wqk_d.ap()[:, :, :])
        wv_sb = const.tile([128, KC, HPC * DH], F32)
        nc.sync.dma_start(wv_sb[:], wv_d.ap()[:, :, :])
        wql_sb = const.tile([128, HPC // 2, 2 * RANK], F32)
        nc.sync.dma_start(wql_sb[:], wql_d.ap()[:, :, :])
        wkl_sb = const.tile([128, HPC // 2, 2 * RANK], F32)
        nc.sync.dma_start(wkl_sb[:], wkl_d.ap()[:, :, :])
        wo_sb = const.tile([128, 2, D], F32)
        nc.sync.dma_start(wo_sb[:], wo_d.ap()[:, :, :])
        mask_sb = const.tile([128, SBW, W], F32)
        nc.sync.dma_start(mask_sb[:], mask_d.ap()[:, :, :])
        ones_sb = const.tile([1, DH], F32)
        nc.vector.memset(ones_sb[:], 1.0)

        xt_dram = xt_d.ap().rearrange("(kc p) t -> p kc t", p=128)

        klt_tiles = [None] * NW
        v_tiles = [None] * NSB

        for w in range(NW):
            # ---------------- QKV projection for window w ----------------
            xt_w = xt_pool.tile([128, KC, W], F32, tag="xt")
            nc.sync.dma_start(xt_w[:], xt_dram[:, :, w * W:(w + 1) * W])

            qt_w = qkt_pool.tile([128, 2, W], F32, tag="qt")
            kt_w = qkt_pool.tile([128, 2, W], F32, tag="kt")
            for pc in range(4):  # 0,1 = q head-pairs; 2,3 = k head-pairs
                ps = acc.tile([128, W], F32, tag="acc")
                for kc in range(KC):
                    nc.tensor.matmul(
                        ps[:],
                        mm(wqk_sb[:, kc, pc * 128:(pc + 1) * 128]),
                        mm(xt_w[:, kc, :]),
                        start=(kc == 0), stop=(kc == KC - 1),
                    )
                dst = (qt_w if pc < 2 else kt_w)[:, pc % 2, :]
                nc.vector.tensor_copy(dst, ps[:])

            for tl in range(SBW):
                sb = w * SBW + tl
                ps = acc.tile([128, HPC * DH], F32, tag="acc")
                for kc in range(KC):
                    nc.tensor.matmul(
                        ps[:],
                        mm(xt_w[:, kc, tl * 128:(tl + 1) * 128]),
                        mm(wv_sb[:, kc, :]),
                        start=(kc == 0), stop=(kc == KC - 1),
                    )
                vt = v_pool.tile([128, HPC, DH + 1], F32, tag="v")
                nc.vector.tensor_copy(
                    vt[:, :, 0:DH], ps[:].rearrange("p (h d) -> p h d", h=HPC)
                )
                nc.vector.memset(vt[:, :, DH:DH + 1], 1.0)
                v_tiles[sb] = vt

            # ---------------- low-rank ql^T / kl^T for window w ----------------
            qlt_w = qlt_pool.tile([128, W], F32, tag="qlt")
            klt_w = klt_pool.tile([128, W], F32, tag="klt")
            for pair in range(2):
                ps = acc.tile([64, W], F32, tag="acc")
                nc.tensor.matmul(ps[:], mm(wql_sb[:, pair, :]), mm(qt_w[:, pair, :]),
                                 start=True, stop=True)
                nc.vector.tensor_copy(qlt_w[pair * 64:(pair + 1) * 64, :], ps[:])
                ps = acc.tile([64, W], F32, tag="acc")
                nc.tensor.matmul(ps[:], mm(wkl_sb[:, pair, :]), mm(kt_w[:, pair, :]),
                                 start=True, stop=True)
                nc.vector.tensor_copy(klt_w[pair * 64:(pair + 1) * 64, :], ps[:])
            klt_tiles[w] = klt_w

            # ---------------- attention for window w ----------------
            y_w = y_pool.tile([128, 2, W], F32, tag="y")
            n_sb = (w + 1) * SBW
            for h in range(HPC):
                yps = acc.tile([DH + 1, W], F32, tag="acc")
                for sb in range(n_sb):
                    sp = stp.tile([128, W], F32, tag="st")
                    nc.tensor.matmul(
                        sp[:],
                        mm(klt_tiles[sb // SBW][32 * h:32 * h + 32,
                                                (sb % SBW) * 128:(sb % SBW + 1) * 128]),
                        mm(qlt_w[32 * h:32 * h + 32, :]),
                        start=True, stop=True,
                        tile_position=(32 * h, 0),
                    )
                    pt = p_pool.tile([128, W], F32, tag="pt")
                    nc.scalar.activation(pt[:], sp[:],
                                         mybir.ActivationFunctionType.Exp)
                    if sb // SBW == w:  # diagonal window -> causal mask
                        nc.vector.tensor_mul(pt[:], pt[:], mask_sb[:, sb % SBW, :])
                    nc.tensor.matmul(
                        yps[:], mm(v_tiles[sb][:, h, :]), mm(pt[:]),
                        start=(sb == 0), stop=(sb == n_sb - 1),
                    )
                den = dr_pool.tile([1, W], F32, tag="den")
                nc.vector.tensor_copy(den[:], yps[DH:DH + 1, :])
                rec = dr_pool.tile([1, W], F32, tag="rec")
                nc.vector.reciprocal(rec[:], den[:])
                bc = stp.tile([64, W], F32, tag="st")
                nc.tensor.matmul(bc[:], mm(ones_sb[0:1, :]), mm(rec[0:1, :]),
                                 start=True, stop=True)
                bc_sb = dr_pool.tile([64, W], F32, tag="bcs")
                nc.vector.tensor_copy(bc_sb[:], bc[:])
                nc.vector.tensor_mul(
                    y_w[(h % 2) * 64:(h % 2) * 64 + 64, h // 2, :],
                    yps[0:DH, :], bc_sb[:],
                )

            # ---------------- W_o projection for window w ----------------
            for tl in range(SBW):
                stg = stage_pool.tile([128, D], F32, tag="stg")
                for nch in range(D // W):
                    ps = acc.tile([128, W], F32, tag="acc")
                    for kc2 in range(2):
                        nc.tensor.matmul(
                            ps[:],
                            mm(y_w[:, kc2, tl * 128:(tl + 1) * 128]),
                            mm(wo_sb[:, kc2, nch * W:(nch + 1) * W]),
                            start=(kc2 == 0), stop=(kc2 == 1),
                        )
                    nc.vector.tensor_copy(stg[:, nch * W:(nch + 1) * W], ps[:])
                r0 = w * W + tl * 128
                nc.sync.dma_start(out_d.ap()[r0:r0 + 128, :], stg[:])

    nc.compile()
    return nc


def _get_nc(t_len=T, mm_dtype=F32):
    key = (t_len, mm_dtype)
    if key not in _NC_CACHE:
        _NC_CACHE[key] = _build_nc(t_len, mm_dtype)
    return _NC_CACHE[key]


def _build_masks(t_len=T):
    SBW = W // 128
    p = np.arange(128)[:, None]
    c = np.arange(W)[None, :]
    masks = np.zeros((128, SBW, W), np.float32)
    for m in range(SBW):
        masks[:, m, :] = (c - p >= 128 * m).astype(np.float32)
    return masks


def make_in_maps(x, W_qkv, W_q_lsr, W_k_lsr, lsr_core, W_o, t_len=T):
    x = np.asarray(x, np.float32)
    W_qkv = np.asarray(W_qkv, np.float32)
    W_q_lsr = np.asarray(W_q_lsr, np.float32)
    W_k_lsr = np.asarray(W_k_lsr, np.float32)
    lsr_core = np.asarray(lsr_core, np.float32)
    W_o = np.asarray(W_o, np.float32)

    d = x.shape[-1]
    masks = _build_masks(t_len)
    scale = lsr_core / np.sqrt(np.float32(RANK))  # [H, R]

    in_maps = []
    for i in range(N_CORES):
        b = i // CPB
        hg = i % CPB
        h0 = hg * HPC
        col0 = h0 * DH
        wq = W_qkv[:, col0:col0 + HPC * DH]
        wk = W_qkv[:, d + col0:d + col0 + HPC * DH]
        wv = W_qkv[:, 2 * d + col0:2 * d + col0 + HPC * DH]
        wqk = np.concatenate([wq, wk], axis=1).reshape(KC, 128, 2 * HPC * DH)
        wqk = wqk.transpose(1, 0, 2)
        wv_p = wv.reshape(KC, 128, HPC * DH).transpose(1, 0, 2)
        wql = np.zeros((HPC // 2, 128, 2 * RANK), np.float32)
        wkl = np.zeros((HPC // 2, 128, 2 * RANK), np.float32)
        for pair in range(HPC // 2):
            for k in range(2):
                h = h0 + pair * 2 + k
                wql[pair, k * DH:(k + 1) * DH, k * RANK:(k + 1) * RANK] = (
                    W_q_lsr[h] * scale[h][None, :]
                )
                wkl[pair, k * DH:(k + 1) * DH, k * RANK:(k + 1) * RANK] = W_k_lsr[h]
        wo = W_o[col0:col0 + HPC * DH, :].reshape(2, 128, d)
        in_maps.append({
            "xt": np.ascontiguousarray(x[b, :t_len].T),
            "wqk": np.ascontiguousarray(wqk),
            "wv": np.ascontiguousarray(wv_p),
            "wql": np.ascontiguousarray(wql.transpose(1, 0, 2)),
            "wkl": np.ascontiguousarray(wkl.transpose(1, 0, 2)),
            "wo": np.ascontiguousarray(wo.transpose(1, 0, 2)),
            "masks": masks,
        })
    return in_maps


def gather_output(results, t_len=T):
    outs = [np.asarray(results[i]["out"], np.float32) for i in range(N_CORES)]
    full = np.stack(
        [sum(outs[b * CPB:(b + 1) * CPB]) for b in range(B)], axis=0
    )
    return full.astype(np.float32)


def run_sharded(inputs, trace=False, t_len=T, mm_dtype=F32):
    nc = _get_nc(t_len, mm_dtype)
    in_maps = make_in_maps(t_len=t_len, **inputs)
    res = run_bass_kernel_spmd(nc, in_maps, core_ids=list(range(N_CORES)),
                               trace=trace)
    return gather_output(res.results, t_len), res


def kernel(x, W_qkv, W_q_lsr, W_k_lsr, lsr_core, W_o):
    out, _ = run_sharded(dict(x=x, W_qkv=W_qkv, W_q_lsr=W_q_lsr,
                              W_k_lsr=W_k_lsr, lsr_core=lsr_core, W_o=W_o))
    return out


# revision 3
# speedup vs baseline: 1.8233x; 1.8233x over previous
"""Multi-head self-attention with low-rank bilinear scores (LSR) on 8 TRN2 cores.

Sharding: core i handles batch b = i//4 and 4 heads (i%4)*4..+4.
Per core, everything runs in the "transposed" orientation:
  qkv^T = W^T @ x^T   (contract over D, x pre-transposed on host)
  ql^T / kl^T via block-diagonal packed LSR weights (2 heads per matmul)
  scores^T [s, t] = kl^T-slices @ ql^T  (K = RANK = 32, per-head 32-row strips)
  P = exp(scores^T) * causal_mask;  PV uses V with an appended ones column so
  the softmax denominator falls out of the same matmul (row 64 of y_psum).
  y^T gets normalized by 1/den (broadcast via a K=1 ones matmul), then the
  W_o row-slice projection produces a per-core partial [T, D] output.
Host: per-batch sum over the 4 cores sharing that batch (W_o is row-sharded),
then stack the two batches.
"""

import numpy as np
from contextlib import ExitStack

import concourse.tile as tile
from concourse import bacc, mybir
from concourse.bass_utils import run_bass_kernel_spmd

B, T, D = 2, 2048, 1024
H, RANK = 16, 32
DH = D // H  # 64
N_CORES = 8
HPC = 4  # heads per core
CPB = N_CORES // B  # cores per batch
W = 512  # t-window (= one fp32 PSUM bank)
KC = D // 128  # k-chunks over D
F32 = mybir.dt.float32
BF16 = mybir.dt.bfloat16
F32R = mybir.dt.float32r

MM_DTYPE = BF16  # dtype of all TensorE-facing operands (except the tiny
                 # fp32 denominator-broadcast matmul)

_NC_CACHE = {}


def _build_nc(t_len=T, mm_dtype=MM_DTYPE):
    """Build + compile the single-core SPMD program (same NEFF on all cores)."""
    NW = t_len // W  # windows
    SBW = W // 128  # s-chunks per window
    NSB = t_len // 128  # total s-chunks
    DT = mm_dtype

    nc = bacc.Bacc("TRN2", target_bir_lowering=False, debug=False,
                   num_devices=N_CORES)

    xt_d = nc.dram_tensor("xt", [D, t_len], DT, kind="ExternalInput")
    wqk_d = nc.dram_tensor("wqk", [128, KC, 2 * HPC * DH], DT, kind="ExternalInput")
    wv_d = nc.dram_tensor("wv", [128, KC, HPC * DH], DT, kind="ExternalInput")
    wql_d = nc.dram_tensor("wql", [128, HPC // 2, 2 * RANK], DT, kind="ExternalInput")
    wkl_d = nc.dram_tensor("wkl", [128, HPC // 2, 2 * RANK], DT, kind="ExternalInput")
    wo_d = nc.dram_tensor("wo", [128, 2, D], DT, kind="ExternalInput")
    mask_d = nc.dram_tensor("masks", [128, SBW, W], DT, kind="ExternalInput")
    out_d = nc.dram_tensor("out", [t_len, D], F32, kind="ExternalOutput")

    with tile.TileContext(nc) as tc, ExitStack() as ctx:
        const = ctx.enter_context(tc.tile_pool(name="const", bufs=1))
        xt_pool = ctx.enter_context(tc.tile_pool(name="xtp", bufs=2))
        qkt_pool = ctx.enter_context(tc.tile_pool(name="qktp", bufs=2))
        klt_pool = ctx.enter_context(tc.tile_pool(name="kltp", bufs=NW))
        qlt_pool = ctx.enter_context(tc.tile_pool(name="qltp", bufs=2))
        v_pool = ctx.enter_context(tc.tile_pool(name="vp", bufs=NSB))
        p_pool = ctx.enter_context(tc.tile_pool(name="pp", bufs=3))
        y_pool = ctx.enter_context(tc.tile_pool(name="yp", bufs=2))
        dr_pool = ctx.enter_context(tc.tile_pool(name="drp", bufs=2))
        stage_pool = ctx.enter_context(tc.tile_pool(name="stgp", bufs=2))
        acc = ctx.enter_context(tc.tile_pool(name="accp", bufs=4, space="PSUM"))
        stp = ctx.enter_context(tc.tile_pool(name="stpp", bufs=4, space="PSUM"))

        wqk_sb = const.tile([128, KC, 2 * HPC * DH], DT)
        nc.sync.dma_start(wqk_sb[:], wqk_d.ap()[:, :, :])
        wv_sb = const.tile([128, KC, HPC * DH], DT)
        nc.sync.dma_start(wv_sb[:], wv_d.ap()[:, :, :])
        wql_sb = const.tile([128, HPC // 2, 2 * RANK], DT)
        nc.sync.dma_start(wql_sb[:], wql_d.ap()[:, :, :])
        wkl_sb = const.tile([128, HPC // 2, 2 * RANK], DT)
        nc.sync.dma_start(wkl_sb[:], wkl_d.ap()[:, :, :])
        wo_sb = const.tile([128, 2, D], DT)
        nc.sync.dma_start(wo_sb[:], wo_d.ap()[:, :, :])
        mask_sb = const.tile([128, SBW, W], DT)
        nc.sync.dma_start(mask_sb[:], mask_d.ap()[:, :, :])
        ones_sb = const.tile([1, DH], F32)
        nc.vector.memset(ones_sb[:], 1.0)

        xt_dram = xt_d.ap().rearrange("(kc p) t -> p kc t", p=128)

        klt_tiles = [None] * NW
        v_tiles = [None] * NSB

        for w in range(NW):
            # ---------------- QKV projection for window w ----------------
            xt_w = xt_pool.tile([128, KC, W], DT, tag="xt")
            nc.sync.dma_start(xt_w[:], xt_dram[:, :, w * W:(w + 1) * W])

            qt_w = qkt_pool.tile([128, 2, W], DT, tag="qt")
            kt_w = qkt_pool.tile([128, 2, W], DT, tag="kt")
            for pc in range(4):  # 0,1 = q head-pairs; 2,3 = k head-pairs
                ps = acc.tile([128, W], F32, tag="acc")
                for kc in range(KC):
                    nc.tensor.matmul(
                        ps[:],
                        wqk_sb[:, kc, pc * 128:(pc + 1) * 128],
                        xt_w[:, kc, :],
                        start=(kc == 0), stop=(kc == KC - 1),
                    )
                dst = (qt_w if pc < 2 else kt_w)[:, pc % 2, :]
                nc.vector.tensor_copy(dst, ps[:])

            for tl in range(SBW):
                sb = w * SBW + tl
                ps = acc.tile([128, HPC * DH], F32, tag="acc")
                for kc in range(KC):
                    nc.tensor.matmul(
                        ps[:],
                        xt_w[:, kc, tl * 128:(tl + 1) * 128],
                        wv_sb[:, kc, :],
                        start=(kc == 0), stop=(kc == KC - 1),
                    )
                vt = v_pool.tile([128, HPC, DH + 1], DT, tag="v")
                nc.vector.tensor_copy(
                    vt[:, :, 0:DH], ps[:].rearrange("p (h d) -> p h d", h=HPC)
                )
                nc.vector.memset(vt[:, :, DH:DH + 1], 1.0)
                v_tiles[sb] = vt

            # ---------------- low-rank ql^T / kl^T for window w ----------------
            qlt_w = qlt_pool.tile([128, W], DT, tag="qlt")
            klt_w = klt_pool.tile([128, W], DT, tag="klt")
            for pair in range(2):
                ps = acc.tile([64, W], F32, tag="acc")
                nc.tensor.matmul(ps[:], wql_sb[:, pair, :], qt_w[:, pair, :],
                                 start=True, stop=True)
                nc.vector.tensor_copy(qlt_w[pair * 64:(pair + 1) * 64, :], ps[:])
                ps = acc.tile([64, W], F32, tag="acc")
                nc.tensor.matmul(ps[:], wkl_sb[:, pair, :], kt_w[:, pair, :],
                                 start=True, stop=True)
                nc.vector.tensor_copy(klt_w[pair * 64:(pair + 1) * 64, :], ps[:])
            klt_tiles[w] = klt_w

            # ---------------- attention for window w ----------------
            y_w = y_pool.tile([128, 2, W], DT, tag="y")
            n_sb = (w + 1) * SBW
            for h in range(HPC):
                yps = acc.tile([DH + 1, W], F32, tag="acc")
                for sb in range(n_sb):
                    sp = stp.tile([128, W], F32, tag="st")
                    nc.tensor.matmul(
                        sp[:],
                        klt_tiles[sb // SBW][32 * h:32 * h + 32,
                                             (sb % SBW) * 128:(sb % SBW + 1) * 128],
                        qlt_w[32 * h:32 * h + 32, :],
                        start=True, stop=True,
                        tile_position=(32 * h, 0),
                    )
                    pt = p_pool.tile([128, W], DT, tag="pt")
                    nc.scalar.activation(pt[:], sp[:],
                                         mybir.ActivationFunctionType.Exp)
                    if sb // SBW == w:  # diagonal window -> causal mask
                        nc.vector.tensor_mul(pt[:], pt[:], mask_sb[:, sb % SBW, :])
                    nc.tensor.matmul(
                        yps[:], v_tiles[sb][:, h, :], pt[:],
                        start=(sb == 0), stop=(sb == n_sb - 1),
                    )
                den = dr_pool.tile([1, W], F32, tag="den")
                nc.vector.tensor_copy(den[:], yps[DH:DH + 1, :])
                rec = dr_pool.tile([1, W], F32, tag="rec")
                nc.vector.reciprocal(rec[:], den[:])
                bc = stp.tile([64, W], F32, tag="st")
                nc.tensor.matmul(bc[:], ones_sb[0:1, :], rec[0:1, :],
                                 start=True, stop=True)
                bc_sb = dr_pool.tile([64, W], F32, tag="bcs")
                nc.vector.tensor_copy(bc_sb[:], bc[:])
                nc.vector.tensor_mul(
                    y_w[(h % 2) * 64:(h % 2) * 64 + 64, h // 2, :],
                    yps[0:DH, :], bc_sb[:],
                )

            # ---------------- W_o projection for window w ----------------
            for tl in range(SBW):
                stg = stage_pool.tile([128, D], F32, tag="stg")
                for nch in range(D // W):
                    ps = acc.tile([128, W], F32, tag="acc")
                    for kc2 in range(2):
                        nc.tensor.matmul(
                            ps[:],
                            y_w[:, kc2, tl * 128:(tl + 1) * 128],
                            wo_sb[:, kc2, nch * W:(nch + 1) * W],
                            start=(kc2 == 0), stop=(kc2 == 1),
                        )
                    nc.vector.tensor_copy(stg[:, nch * W:(nch + 1) * W], ps[:])
                r0 = w * W + tl * 128
                nc.sync.dma_start(out_d.ap()[r0:r0 + 128, :], stg[:])

    nc.compile()
    return nc


def _get_nc(t_len=T, mm_dtype=MM_DTYPE):
    key = (t_len, mm_dtype)
    if key not in _NC_CACHE:
        _NC_CACHE[key] = _build_nc(t_len, mm_dtype)
    return _NC_CACHE[key]


def _np_dt(mm_dtype):
    return mybir.dt.np(mm_dtype)


def _build_masks(t_len=T):
    SBW = W // 128
    p = np.arange(128)[:, None]
    c = np.arange(W)[None, :]
    masks = np.zeros((128, SBW, W), np.float32)
    for m in range(SBW):
        masks[:, m, :] = (c - p >= 128 * m).astype(np.float32)
    return masks


def make_in_maps(x, W_qkv, W_q_lsr, W_k_lsr, lsr_core, W_o, t_len=T,
                 mm_dtype=MM_DTYPE):
    x = np.asarray(x, np.float32)
    W_qkv = np.asarray(W_qkv, np.float32)
    W_q_lsr = np.asarray(W_q_lsr, np.float32)
    W_k_lsr = np.asarray(W_k_lsr, np.float32)
    lsr_core = np.asarray(lsr_core, np.float32)
    W_o = np.asarray(W_o, np.float32)
    ndt = _np_dt(mm_dtype)

    d = x.shape[-1]
    masks = _build_masks(t_len).astype(ndt)
    scale = lsr_core / np.sqrt(np.float32(RANK))  # [H, R]

    in_maps = []
    for i in range(N_CORES):
        b = i // CPB
        hg = i % CPB
        h0 = hg * HPC
        col0 = h0 * DH
        wq = W_qkv[:, col0:col0 + HPC * DH]
        wk = W_qkv[:, d + col0:d + col0 + HPC * DH]
        wv = W_qkv[:, 2 * d + col0:2 * d + col0 + HPC * DH]
        wqk = np.concatenate([wq, wk], axis=1).reshape(KC, 128, 2 * HPC * DH)
        wqk = wqk.transpose(1, 0, 2)
        wv_p = wv.reshape(KC, 128, HPC * DH).transpose(1, 0, 2)
        wql = np.zeros((HPC // 2, 128, 2 * RANK), np.float32)
        wkl = np.zeros((HPC // 2, 128, 2 * RANK), np.float32)
        for pair in range(HPC // 2):
            for k in range(2):
                h = h0 + pair * 2 + k
                wql[pair, k * DH:(k + 1) * DH, k * RANK:(k + 1) * RANK] = (
                    W_q_lsr[h] * scale[h][None, :]
                )
                wkl[pair, k * DH:(k + 1) * DH, k * RANK:(k + 1) * RANK] = W_k_lsr[h]
        wo = W_o[col0:col0 + HPC * DH, :].reshape(2, 128, d)
        in_maps.append({
            "xt": np.ascontiguousarray(x[b, :t_len].T).astype(ndt),
            "wqk": np.ascontiguousarray(wqk).astype(ndt),
            "wv": np.ascontiguousarray(wv_p).astype(ndt),
            "wql": np.ascontiguousarray(wql.transpose(1, 0, 2)).astype(ndt),
            "wkl": np.ascontiguousarray(wkl.transpose(1, 0, 2)).astype(ndt),
            "wo": np.ascontiguousarray(wo.transpose(1, 0, 2)).astype(ndt),
            "masks": masks,
        })
    return in_maps


def gather_output(results, t_len=T):
    outs = [np.asarray(results[i]["out"], np.float32) for i in range(N_CORES)]
    full = np.stack(
        [sum(outs[b * CPB:(b + 1) * CPB]) for b in range(B)], axis=0
    )
    return full.astype(np.float32)


def run_sharded(inputs, trace=False, t_len=T, mm_dtype=MM_DTYPE):
    nc = _get_nc(t_len, mm_dtype)
    in_maps = make_in_maps(t_len=t_len, mm_dtype=mm_dtype, **inputs)
    res = run_bass_kernel_spmd(nc, in_maps, core_ids=list(range(N_CORES)),
                               trace=trace)
    return gather_output(res.results, t_len), res


def kernel(x, W_qkv, W_q_lsr, W_k_lsr, lsr_core, W_o):
    out, _ = run_sharded(dict(x=x, W_qkv=W_qkv, W_q_lsr=W_q_lsr,
                              W_k_lsr=W_k_lsr, lsr_core=lsr_core, W_o=W_o))
    return out


# revision 7
# speedup vs baseline: 2.6280x; 1.4413x over previous
"""Multi-head self-attention with low-rank bilinear scores (LSR) on 8 TRN2 cores.

Sharding: core i handles batch b = i//4 and 4 heads (i%4)*4..+4.
Per core, everything runs in the "transposed" orientation:
  qkv^T = W^T @ x^T   (contract over D, x pre-transposed on host)
  ql^T / kl^T via block-diagonal packed LSR weights (2 heads per matmul)
  scores^T [s, t] = kl^T-slices @ ql^T  (K = RANK = 32, per-head 32-row strips)
  P = exp(scores^T) * causal_mask;  PV uses V with an appended ones column so
  the softmax denominator falls out of the same matmul (row 64 of y_psum).
  y^T gets normalized by 1/den (broadcast via a K=1 ones matmul), then the
  W_o row-slice projection produces a per-core partial [T, D] output.
Host: per-batch sum over the 4 cores sharing that batch (W_o is row-sharded),
then stack the two batches.
"""

import numpy as np
from contextlib import ExitStack

import concourse.tile as tile
from concourse import bacc, mybir
from concourse.bass_utils import run_bass_kernel_spmd

B, T, D = 2, 2048, 1024
H, RANK = 16, 32
DH = D // H  # 64
N_CORES = 8
HPC = 4  # heads per core
CPB = N_CORES // B  # cores per batch
W = 512  # t-window (= one fp32 PSUM bank)
KC = D // 128  # k-chunks over D
F32 = mybir.dt.float32
BF16 = mybir.dt.bfloat16
F32R = mybir.dt.float32r

MM_DTYPE = BF16  # dtype of all TensorE-facing operands (except the tiny
                 # fp32 denominator-broadcast matmul)

_NC_CACHE = {}


def _build_nc(t_len=T, mm_dtype=MM_DTYPE):
    """Build + compile the single-core SPMD program (same NEFF on all cores)."""
    NW = t_len // W  # windows
    SBW = W // 128  # s-chunks per window
    NSB = t_len // 128  # total s-chunks
    DT = mm_dtype

    nc = bacc.Bacc("TRN2", target_bir_lowering=False, debug=False,
                   num_devices=N_CORES)

    xt_d = nc.dram_tensor("xt", [D, t_len], DT, kind="ExternalInput")
    wqk_d = nc.dram_tensor("wqk", [128, KC, 2 * HPC * DH], DT, kind="ExternalInput")
    wv_d = nc.dram_tensor("wv", [128, KC, HPC * DH], DT, kind="ExternalInput")
    wql_d = nc.dram_tensor("wql", [128, HPC // 2, 2 * RANK], DT, kind="ExternalInput")
    wkl_d = nc.dram_tensor("wkl", [128, HPC // 2, 2 * RANK], DT, kind="ExternalInput")
    wo_d = nc.dram_tensor("wo", [128, 2, D], DT, kind="ExternalInput")
    mask_d = nc.dram_tensor("masks", [128, SBW, W], DT, kind="ExternalInput")
    out_d = nc.dram_tensor("out", [t_len, D], F32, kind="ExternalOutput")

    with tile.TileContext(nc) as tc, ExitStack() as ctx:
        const = ctx.enter_context(tc.tile_pool(name="const", bufs=1))
        xt_pool = ctx.enter_context(tc.tile_pool(name="xtp", bufs=2))
        qkt_pool = ctx.enter_context(tc.tile_pool(name="qktp", bufs=2))
        klt_pool = ctx.enter_context(tc.tile_pool(name="kltp", bufs=NW))
        qlt_pool = ctx.enter_context(tc.tile_pool(name="qltp", bufs=2))
        v_pool = ctx.enter_context(tc.tile_pool(name="vp", bufs=NSB))
        p_pool = ctx.enter_context(tc.tile_pool(name="pp", bufs=3))
        y_pool = ctx.enter_context(tc.tile_pool(name="yp", bufs=2))
        yun_pool = ctx.enter_context(tc.tile_pool(name="yunp", bufs=2 * HPC))
        dr_pool = ctx.enter_context(tc.tile_pool(name="drp", bufs=4))
        stage_pool = ctx.enter_context(tc.tile_pool(name="stgp", bufs=2))
        acc = ctx.enter_context(tc.tile_pool(name="accp", bufs=4, space="PSUM"))
        stp = ctx.enter_context(tc.tile_pool(name="stpp", bufs=4, space="PSUM"))

        wqk_sb = const.tile([128, KC, 2 * HPC * DH], DT)
        nc.sync.dma_start(wqk_sb[:], wqk_d.ap()[:, :, :])
        wv_sb = const.tile([128, KC, HPC * DH], DT)
        nc.sync.dma_start(wv_sb[:], wv_d.ap()[:, :, :])
        wql_sb = const.tile([128, HPC // 2, 2 * RANK], DT)
        nc.sync.dma_start(wql_sb[:], wql_d.ap()[:, :, :])
        wkl_sb = const.tile([128, HPC // 2, 2 * RANK], DT)
        nc.sync.dma_start(wkl_sb[:], wkl_d.ap()[:, :, :])
        wo_sb = const.tile([128, 2, D], DT)
        nc.sync.dma_start(wo_sb[:], wo_d.ap()[:, :, :])
        mask_sb = const.tile([128, SBW, W], DT)
        nc.sync.dma_start(mask_sb[:], mask_d.ap()[:, :, :])
        ones_sb = const.tile([1, DH], F32)
        nc.vector.memset(ones_sb[:], 1.0)

        xt_dram = xt_d.ap().rearrange("(kc p) t -> p kc t", p=128)

        klt_tiles = [None] * NW
        v_tiles = [None] * NSB

        def norm_and_wo(w, yun_tiles):
            """Normalize window w's per-head y^T and run the W_o projection.

            Emitted one window late so the DVE reciprocal chain never blocks
            the PE instruction stream."""
            y_w = y_pool.tile([128, 2, W], DT, tag="y", name=f"y_{w}")
            for h in range(HPC):
                yun = yun_tiles[h]
                rec = dr_pool.tile([1, W], F32, tag="rec")
                nc.vector.reciprocal(rec[:], yun[DH:DH + 1, :])
                bc = stp.tile([64, W], F32, tag="st")
                nc.tensor.matmul(bc[:], ones_sb[0:1, :], rec[0:1, :],
                                 start=True, stop=True)
                bc_sb = dr_pool.tile([64, W], F32, tag="bcs")
                nc.vector.tensor_copy(bc_sb[:], bc[:])
                nc.vector.tensor_mul(
                    y_w[(h % 2) * 64:(h % 2) * 64 + 64, h // 2, :],
                    yun[0:DH, :], bc_sb[:],
                )
            for tl in range(SBW):
                stg = stage_pool.tile([128, D], F32, tag="stg")
                for nch in range(D // W):
                    ps = acc.tile([128, W], F32, tag="acc")
                    for kc2 in range(2):
                        nc.tensor.matmul(
                            ps[:],
                            y_w[:, kc2, tl * 128:(tl + 1) * 128],
                            wo_sb[:, kc2, nch * W:(nch + 1) * W],
                            start=(kc2 == 0), stop=(kc2 == 1),
                        )
                    nc.vector.tensor_copy(stg[:, nch * W:(nch + 1) * W], ps[:])
                r0 = w * W + tl * 128
                nc.sync.dma_start(out_d.ap()[r0:r0 + 128, :], stg[:])

        pending = None  # (w, yun_tiles) awaiting norm+Wo

        for w in range(NW):
            # ---------------- QKV projection for window w ----------------
            xt_w = xt_pool.tile([128, KC, W], DT, tag="xt")
            nc.sync.dma_start(xt_w[:], xt_dram[:, :, w * W:(w + 1) * W])

            qt_w = qkt_pool.tile([128, 2, W], DT, tag="qt")
            kt_w = qkt_pool.tile([128, 2, W], DT, tag="kt")
            for pc in range(4):  # 0,1 = q head-pairs; 2,3 = k head-pairs
                ps = acc.tile([128, W], F32, tag="acc")
                for kc in range(KC):
                    nc.tensor.matmul(
                        ps[:],
                        wqk_sb[:, kc, pc * 128:(pc + 1) * 128],
                        xt_w[:, kc, :],
                        start=(kc == 0), stop=(kc == KC - 1),
                    )
                dst = (qt_w if pc < 2 else kt_w)[:, pc % 2, :]
                nc.vector.tensor_copy(dst, ps[:])

            for tl in range(SBW):
                sb = w * SBW + tl
                ps = acc.tile([128, HPC * DH], F32, tag="acc")
                for kc in range(KC):
                    nc.tensor.matmul(
                        ps[:],
                        xt_w[:, kc, tl * 128:(tl + 1) * 128],
                        wv_sb[:, kc, :],
                        start=(kc == 0), stop=(kc == KC - 1),
                    )
                vt = v_pool.tile([128, HPC, DH + 1], DT, tag="v")
                nc.vector.tensor_copy(
                    vt[:, :, 0:DH], ps[:].rearrange("p (h d) -> p h d", h=HPC)
                )
                nc.vector.memset(vt[:, :, DH:DH + 1], 1.0)
                v_tiles[sb] = vt

            # ---------------- low-rank ql^T / kl^T for window w ----------------
            qlt_w = qlt_pool.tile([128, W], DT, tag="qlt")
            klt_w = klt_pool.tile([128, W], DT, tag="klt")
            for pair in range(2):
                ps = acc.tile([64, W], F32, tag="acc")
                nc.tensor.matmul(ps[:], wql_sb[:, pair, :], qt_w[:, pair, :],
                                 start=True, stop=True)
                nc.vector.tensor_copy(qlt_w[pair * 64:(pair + 1) * 64, :], ps[:])
                ps = acc.tile([64, W], F32, tag="acc")
                nc.tensor.matmul(ps[:], wkl_sb[:, pair, :], kt_w[:, pair, :],
                                 start=True, stop=True)
                nc.vector.tensor_copy(klt_w[pair * 64:(pair + 1) * 64, :], ps[:])
            klt_tiles[w] = klt_w

            # ---------------- attention for window w ----------------
            n_sb = (w + 1) * SBW
            yun_tiles = []
            for h in range(HPC):
                yps = acc.tile([DH + 1, W], F32, tag="acc")
                for sb in range(n_sb):
                    sp = stp.tile([128, W], F32, tag="st")
                    nc.tensor.matmul(
                        sp[:],
                        klt_tiles[sb // SBW][32 * h:32 * h + 32,
                                             (sb % SBW) * 128:(sb % SBW + 1) * 128],
                        qlt_w[32 * h:32 * h + 32, :],
                        start=True, stop=True,
                        tile_position=(32 * h, 0),
                    )
                    pt = p_pool.tile([128, W], DT, tag="pt")
                    nc.scalar.activation(pt[:], sp[:],
                                         mybir.ActivationFunctionType.Exp)
                    if sb // SBW == w:  # diagonal window -> causal mask
                        nc.vector.tensor_mul(pt[:], pt[:], mask_sb[:, sb % SBW, :])
                    nc.tensor.matmul(
                        yps[:], v_tiles[sb][:, h, :], pt[:],
                        start=(sb == 0), stop=(sb == n_sb - 1),
                    )
                yun = yun_pool.tile([DH + 1, W], F32, tag="yun")
                nc.vector.tensor_copy(yun[:], yps[:])
                yun_tiles.append(yun)

            if pending is not None:
                norm_and_wo(*pending)
            pending = (w, yun_tiles)

        norm_and_wo(*pending)

    nc.compile()
    return nc


def _get_nc(t_len=T, mm_dtype=MM_DTYPE):
    key = (t_len, mm_dtype)
    if key not in _NC_CACHE:
        _NC_CACHE[key] = _build_nc(t_len, mm_dtype)
    return _NC_CACHE[key]


def _np_dt(mm_dtype):
    return mybir.dt.np(mm_dtype)


def _build_masks(t_len=T):
    SBW = W // 128
    p = np.arange(128)[:, None]
    c = np.arange(W)[None, :]
    masks = np.zeros((128, SBW, W), np.float32)
    for m in range(SBW):
        masks[:, m, :] = (c - p >= 128 * m).astype(np.float32)
    return masks


def make_in_maps(x, W_qkv, W_q_lsr, W_k_lsr, lsr_core, W_o, t_len=T,
                 mm_dtype=MM_DTYPE):
    x = np.asarray(x, np.float32)
    W_qkv = np.asarray(W_qkv, np.float32)
    W_q_lsr = np.asarray(W_q_lsr, np.float32)
    W_k_lsr = np.asarray(W_k_lsr, np.float32)
    lsr_core = np.asarray(lsr_core, np.float32)
    W_o = np.asarray(W_o, np.float32)
    ndt = _np_dt(mm_dtype)

    d = x.shape[-1]
    masks = _build_masks(t_len).astype(ndt)
    scale = lsr_core / np.sqrt(np.float32(RANK))  # [H, R]

    in_maps = []
    for i in range(N_CORES):
        b = i // CPB
        hg = i % CPB
        h0 = hg * HPC
        col0 = h0 * DH
        wq = W_qkv[:, col0:col0 + HPC * DH]
        wk = W_qkv[:, d + col0:d + col0 + HPC * DH]
        wv = W_qkv[:, 2 * d + col0:2 * d + col0 + HPC * DH]
        wqk = np.concatenate([wq, wk], axis=1).reshape(KC, 128, 2 * HPC * DH)
        wqk = wqk.transpose(1, 0, 2)
        wv_p = wv.reshape(KC, 128, HPC * DH).transpose(1, 0, 2)
        wql = np.zeros((HPC // 2, 128, 2 * RANK), np.float32)
        wkl = np.zeros((HPC // 2, 128, 2 * RANK), np.float32)
        for pair in range(HPC // 2):
            for k in range(2):
                h = h0 + pair * 2 + k
                wql[pair, k * DH:(k + 1) * DH, k * RANK:(k + 1) * RANK] = (
                    W_q_lsr[h] * scale[h][None, :]
                )
                wkl[pair, k * DH:(k + 1) * DH, k * RANK:(k + 1) * RANK] = W_k_lsr[h]
        wo = W_o[col0:col0 + HPC * DH, :].reshape(2, 128, d)
        in_maps.append({
            "xt": np.ascontiguousarray(x[b, :t_len].T).astype(ndt),
            "wqk": np.ascontiguousarray(wqk).astype(ndt),
            "wv": np.ascontiguousarray(wv_p).astype(ndt),
            "wql": np.ascontiguousarray(wql.transpose(1, 0, 2)).astype(ndt),
            "wkl": np.ascontiguousarray(wkl.transpose(1, 0, 2)).astype(ndt),
            "wo": np.ascontiguousarray(wo.transpose(1, 0, 2)).astype(ndt),
            "masks": masks,
        })
    return in_maps


def gather_output(results, t_len=T):
    outs = [np.asarray(results[i]["out"], np.float32) for i in range(N_CORES)]
    full = np.stack(
        [sum(outs[b * CPB:(b + 1) * CPB]) for b in range(B)], axis=0
    )
    return full.astype(np.float32)


def run_sharded(inputs, trace=False, t_len=T, mm_dtype=MM_DTYPE):
    nc = _get_nc(t_len, mm_dtype)
    in_maps = make_in_maps(t_len=t_len, mm_dtype=mm_dtype, **inputs)
    res = run_bass_kernel_spmd(nc, in_maps, core_ids=list(range(N_CORES)),
                               trace=trace)
    return gather_output(res.results, t_len), res


def kernel(x, W_qkv, W_q_lsr, W_k_lsr, lsr_core, W_o):
    out, _ = run_sharded(dict(x=x, W_qkv=W_qkv, W_q_lsr=W_q_lsr,
                              W_k_lsr=W_k_lsr, lsr_core=lsr_core, W_o=W_o))
    return out


# revision 10
# speedup vs baseline: 2.7766x; 1.0566x over previous
"""Multi-head self-attention with low-rank bilinear scores (LSR) on 8 TRN2 cores.

Sharding: core i handles batch b = i//4 and 4 heads (i%4)*4..+4.
Per core, everything runs in the "transposed" orientation:
  qkv^T = W^T @ x^T   (contract over D, x pre-transposed on host)
  ql^T / kl^T via block-diagonal packed LSR weights (2 heads per matmul)
  scores^T [s, t] = kl^T-slices @ ql^T  (K = RANK = 32, per-head 32-row strips)
  P = exp(scores^T) * causal_mask;  PV uses V with an appended ones column so
  the softmax denominator falls out of the same matmul (row 64 of y_psum).
  y^T gets normalized by 1/den (broadcast via a K=1 ones matmul), then the
  W_o row-slice projection produces a per-core partial [T, D] output.
Host: per-batch sum over the 4 cores sharing that batch (W_o is row-sharded),
then stack the two batches.
"""

import numpy as np
from contextlib import ExitStack

import concourse.tile as tile
from concourse import bacc, mybir
from concourse.bass_utils import run_bass_kernel_spmd

B, T, D = 2, 2048, 1024
H, RANK = 16, 32
DH = D // H  # 64
N_CORES = 8
HPC = 4  # heads per core
CPB = N_CORES // B  # cores per batch
W = 512  # t-window (= one fp32 PSUM bank)
KC = D // 128  # k-chunks over D
F32 = mybir.dt.float32
BF16 = mybir.dt.bfloat16
F32R = mybir.dt.float32r

MM_DTYPE = BF16  # dtype of all TensorE-facing operands (except the tiny
                 # fp32 denominator-broadcast matmul)

_NC_CACHE = {}


def _build_nc(t_len=T, mm_dtype=MM_DTYPE):
    """Build + compile the single-core SPMD program (same NEFF on all cores)."""
    NW = t_len // W  # windows
    SBW = W // 128  # s-chunks per window
    NSB = t_len // 128  # total s-chunks
    DT = mm_dtype

    nc = bacc.Bacc("TRN2", target_bir_lowering=False, debug=False,
                   num_devices=N_CORES)

    xt_d = nc.dram_tensor("xt", [D, t_len], DT, kind="ExternalInput")
    wqk_d = nc.dram_tensor("wqk", [128, KC, 2 * HPC * DH], DT, kind="ExternalInput")
    wv_d = nc.dram_tensor("wv", [128, KC, HPC * DH], DT, kind="ExternalInput")
    wql_d = nc.dram_tensor("wql", [128, HPC // 2, 2 * RANK], DT, kind="ExternalInput")
    wkl_d = nc.dram_tensor("wkl", [128, HPC // 2, 2 * RANK], DT, kind="ExternalInput")
    wo_d = nc.dram_tensor("wo", [128, 2, D], DT, kind="ExternalInput")
    mask_d = nc.dram_tensor("masks", [128, SBW, W], DT, kind="ExternalInput")
    out_d = nc.dram_tensor("out", [t_len, D], F32, kind="ExternalOutput")

    with tile.TileContext(nc) as tc, ExitStack() as ctx:
        const = ctx.enter_context(tc.tile_pool(name="const", bufs=1))
        xt_pool = ctx.enter_context(tc.tile_pool(name="xtp", bufs=2))
        qkt_pool = ctx.enter_context(tc.tile_pool(name="qktp", bufs=2))
        klt_pool = ctx.enter_context(tc.tile_pool(name="kltp", bufs=NW))
        qlt_pool = ctx.enter_context(tc.tile_pool(name="qltp", bufs=2))
        v_pool = ctx.enter_context(tc.tile_pool(name="vp", bufs=NSB))
        p_pool = ctx.enter_context(tc.tile_pool(name="pp", bufs=3))
        y_pool = ctx.enter_context(tc.tile_pool(name="yp", bufs=2))
        yun_pool = ctx.enter_context(tc.tile_pool(name="yunp", bufs=2 * HPC))
        dr_pool = ctx.enter_context(tc.tile_pool(name="drp", bufs=4))
        stage_pool = ctx.enter_context(tc.tile_pool(name="stgp", bufs=2))
        acc = ctx.enter_context(tc.tile_pool(name="accp", bufs=4, space="PSUM"))
        stp = ctx.enter_context(tc.tile_pool(name="stpp", bufs=2, space="PSUM"))

        wqk_sb = const.tile([128, KC, 2 * HPC * DH], DT)
        nc.sync.dma_start(wqk_sb[:], wqk_d.ap()[:, :, :])
        wv_sb = const.tile([128, KC, HPC * DH], DT)
        nc.sync.dma_start(wv_sb[:], wv_d.ap()[:, :, :])
        wql_sb = const.tile([128, HPC // 2, 2 * RANK], DT)
        nc.sync.dma_start(wql_sb[:], wql_d.ap()[:, :, :])
        wkl_sb = const.tile([128, HPC // 2, 2 * RANK], DT)
        nc.sync.dma_start(wkl_sb[:], wkl_d.ap()[:, :, :])
        wo_sb = const.tile([128, 2, D], DT)
        nc.sync.dma_start(wo_sb[:], wo_d.ap()[:, :, :])
        mask_sb = const.tile([128, SBW, W], DT)
        nc.sync.dma_start(mask_sb[:], mask_d.ap()[:, :, :])
        ones_sb = const.tile([1, DH], F32)
        nc.vector.memset(ones_sb[:], 1.0)

        xt_dram = xt_d.ap().rearrange("(kc p) t -> p kc t", p=128)

        klt_tiles = [None] * NW
        v_tiles = [None] * NSB

        def norm_and_wo(w, yun_tiles):
            """Normalize window w's per-head y^T and run the W_o projection.

            Emitted one window late so the DVE reciprocal chain never blocks
            the PE instruction stream."""
            y_w = y_pool.tile([128, 2, W], DT, tag="y", name=f"y_{w}")
            for h in range(HPC):
                yun = yun_tiles[h]
                # 1/den = exp(-ln(den)) on ACT: ~5x faster than the DVE
                # reciprocal and both functions live in one table set.
                lnv = dr_pool.tile([1, W], F32, tag="lnv")
                nc.scalar.activation(lnv[:], yun[DH:DH + 1, :],
                                     mybir.ActivationFunctionType.Ln)
                rec = dr_pool.tile([1, W], F32, tag="rec")
                nc.scalar.activation(rec[:], lnv[:],
                                     mybir.ActivationFunctionType.Exp,
                                     scale=-1.0)
                bc = acc.tile([64, W], F32, tag="acc")
                nc.tensor.matmul(bc[:], ones_sb[0:1, :], rec[0:1, :],
                                 start=True, stop=True)
                bc_sb = dr_pool.tile([64, W], F32, tag="bcs")
                nc.vector.tensor_copy(bc_sb[:], bc[:])
                nc.vector.tensor_mul(
                    y_w[(h % 2) * 64:(h % 2) * 64 + 64, h // 2, :],
                    yun[0:DH, :], bc_sb[:],
                )
            for tl in range(SBW):
                stg = stage_pool.tile([128, D], F32, tag="stg")
                for nch in range(D // W):
                    ps = acc.tile([128, W], F32, tag="acc")
                    for kc2 in range(2):
                        nc.tensor.matmul(
                            ps[:],
                            y_w[:, kc2, tl * 128:(tl + 1) * 128],
                            wo_sb[:, kc2, nch * W:(nch + 1) * W],
                            start=(kc2 == 0), stop=(kc2 == 1),
                        )
                    nc.vector.tensor_copy(stg[:, nch * W:(nch + 1) * W], ps[:])
                r0 = w * W + tl * 128
                nc.sync.dma_start(out_d.ap()[r0:r0 + 128, :], stg[:])

        pending = None  # (w, yun_tiles) awaiting norm+Wo

        for w in range(NW):
            # ---------------- QKV projection for window w ----------------
            xt_w = xt_pool.tile([128, KC, W], DT, tag="xt")
            nc.sync.dma_start(xt_w[:], xt_dram[:, :, w * W:(w + 1) * W])

            qt_w = qkt_pool.tile([128, 2, W], DT, tag="qt")
            kt_w = qkt_pool.tile([128, 2, W], DT, tag="kt")
            for pc in range(4):  # 0,1 = q head-pairs; 2,3 = k head-pairs
                ps = acc.tile([128, W], F32, tag="acc")
                for kc in range(KC):
                    nc.tensor.matmul(
                        ps[:],
                        wqk_sb[:, kc, pc * 128:(pc + 1) * 128],
                        xt_w[:, kc, :],
                        start=(kc == 0), stop=(kc == KC - 1),
                    )
                dst = (qt_w if pc < 2 else kt_w)[:, pc % 2, :]
                nc.vector.tensor_copy(dst, ps[:])

            for tl in range(SBW):
                sb = w * SBW + tl
                ps = acc.tile([128, HPC * DH], F32, tag="acc")
                for kc in range(KC):
                    nc.tensor.matmul(
                        ps[:],
                        xt_w[:, kc, tl * 128:(tl + 1) * 128],
                        wv_sb[:, kc, :],
                        start=(kc == 0), stop=(kc == KC - 1),
                    )
                vt = v_pool.tile([128, HPC, DH + 1], DT, tag="v")
                nc.vector.tensor_copy(
                    vt[:, :, 0:DH], ps[:].rearrange("p (h d) -> p h d", h=HPC)
                )
                nc.vector.memset(vt[:, :, DH:DH + 1], 1.0)
                v_tiles[sb] = vt

            # ---------------- low-rank ql^T / kl^T for window w ----------------
            qlt_w = qlt_pool.tile([128, W], DT, tag="qlt")
            klt_w = klt_pool.tile([128, W], DT, tag="klt")
            for pair in range(2):
                ps = acc.tile([64, W], F32, tag="acc")
                nc.tensor.matmul(ps[:], wql_sb[:, pair, :], qt_w[:, pair, :],
                                 start=True, stop=True)
                nc.vector.tensor_copy(qlt_w[pair * 64:(pair + 1) * 64, :], ps[:])
                ps = acc.tile([64, W], F32, tag="acc")
                nc.tensor.matmul(ps[:], wkl_sb[:, pair, :], kt_w[:, pair, :],
                                 start=True, stop=True)
                nc.vector.tensor_copy(klt_w[pair * 64:(pair + 1) * 64, :], ps[:])
            klt_tiles[w] = klt_w

            # ---------------- attention for window w ----------------
            # s-chunks are processed in pairs: one [128, 2, W] scores psum
            # tile (2 banks), a single exp over both halves, and a 1-pair
            # PE lookahead so the scores matmuls of pair i+1 run while ACT
            # computes exp of pair i (the attention phase is ACT-bound).
            n_sb = (w + 1) * SBW
            npairs = n_sb // 2
            yun_tiles = []
            for h in range(HPC):
                yps = acc.tile([DH + 1, W], F32, tag="acc")
                pt_tiles = {}

                def emit_st_exp(i, h=h, pt_tiles=pt_tiles, qlt_w=qlt_w, w=w):
                    sp = stp.tile([128, 2, W], F32, tag="st")
                    for j in (0, 1):
                        sb = 2 * i + j
                        nc.tensor.matmul(
                            sp[:, j, :],
                            klt_tiles[sb // SBW][32 * h:32 * h + 32,
                                                 (sb % SBW) * 128:(sb % SBW + 1) * 128],
                            qlt_w[32 * h:32 * h + 32, :],
                            start=True, stop=True,
                            tile_position=(32 * h, 0),
                        )
                    pt = p_pool.tile([128, 2, W], DT, tag="pt")
                    nc.scalar.activation(pt[:], sp[:],
                                         mybir.ActivationFunctionType.Exp)
                    if (2 * i) // SBW == w:  # diagonal window -> causal mask
                        m = (2 * i) % SBW
                        nc.vector.tensor_mul(pt[:], pt[:], mask_sb[:, m:m + 2, :])
                    pt_tiles[i] = pt

                emit_st_exp(0)
                for i in range(npairs):
                    if i + 1 < npairs:
                        emit_st_exp(i + 1)
                    pt = pt_tiles.pop(i)
                    for j in (0, 1):
                        sb = 2 * i + j
                        nc.tensor.matmul(
                            yps[:], v_tiles[sb][:, h, :], pt[:, j, :],
                            start=(sb == 0), stop=(sb == n_sb - 1),
                        )
                yun = yun_pool.tile([DH + 1, W], F32, tag="yun")
                nc.vector.tensor_copy(yun[:], yps[:])
                yun_tiles.append(yun)

            if pending is not None:
                norm_and_wo(*pending)
            pending = (w, yun_tiles)

        norm_and_wo(*pending)

    nc.compile()
    return nc


def _get_nc(t_len=T, mm_dtype=MM_DTYPE):
    key = (t_len, mm_dtype)
    if key not in _NC_CACHE:
        _NC_CACHE[key] = _build_nc(t_len, mm_dtype)
    return _NC_CACHE[key]


def _np_dt(mm_dtype):
    return mybir.dt.np(mm_dtype)


def _build_masks(t_len=T):
    SBW = W // 128
    p = np.arange(128)[:, None]
    c = np.arange(W)[None, :]
    masks = np.zeros((128, SBW, W), np.float32)
    for m in range(SBW):
        masks[:, m, :] = (c - p >= 128 * m).astype(np.float32)
    return masks


def make_in_maps(x, W_qkv, W_q_lsr, W_k_lsr, lsr_core, W_o, t_len=T,
                 mm_dtype=MM_DTYPE):
    x = np.asarray(x, np.float32)
    W_qkv = np.asarray(W_qkv, np.float32)
    W_q_lsr = np.asarray(W_q_lsr, np.float32)
    W_k_lsr = np.asarray(W_k_lsr, np.float32)
    lsr_core = np.asarray(lsr_core, np.float32)
    W_o = np.asarray(W_o, np.float32)
    ndt = _np_dt(mm_dtype)

    d = x.shape[-1]
    masks = _build_masks(t_len).astype(ndt)
    scale = lsr_core / np.sqrt(np.float32(RANK))  # [H, R]

    in_maps = []
    for i in range(N_CORES):
        b = i // CPB
        hg = i % CPB
        h0 = hg * HPC
        col0 = h0 * DH
        wq = W_qkv[:, col0:col0 + HPC * DH]
        wk = W_qkv[:, d + col0:d + col0 + HPC * DH]
        wv = W_qkv[:, 2 * d + col0:2 * d + col0 + HPC * DH]
        wqk = np.concatenate([wq, wk], axis=1).reshape(KC, 128, 2 * HPC * DH)
        wqk = wqk.transpose(1, 0, 2)
        wv_p = wv.reshape(KC, 128, HPC * DH).transpose(1, 0, 2)
        wql = np.zeros((HPC // 2, 128, 2 * RANK), np.float32)
        wkl = np.zeros((HPC // 2, 128, 2 * RANK), np.float32)
        for pair in range(HPC // 2):
            for k in range(2):
                h = h0 + pair * 2 + k
                wql[pair, k * DH:(k + 1) * DH, k * RANK:(k + 1) * RANK] = (
                    W_q_lsr[h] * scale[h][None, :]
                )
                wkl[pair, k * DH:(k + 1) * DH, k * RANK:(k + 1) * RANK] = W_k_lsr[h]
        wo = W_o[col0:col0 + HPC * DH, :].reshape(2, 128, d)
        in_maps.append({
            "xt": np.ascontiguousarray(x[b, :t_len].T).astype(ndt),
            "wqk": np.ascontiguousarray(wqk).astype(ndt),
            "wv": np.ascontiguousarray(wv_p).astype(ndt),
            "wql": np.ascontiguousarray(wql.transpose(1, 0, 2)).astype(ndt),
            "wkl": np.ascontiguousarray(wkl.transpose(1, 0, 2)).astype(ndt),
            "wo": np.ascontiguousarray(wo.transpose(1, 0, 2)).astype(ndt),
            "masks": masks,
        })
    return in_maps


def gather_output(results, t_len=T):
    outs = [np.asarray(results[i]["out"], np.float32) for i in range(N_CORES)]
    full = np.stack(
        [sum(outs[b * CPB:(b + 1) * CPB]) for b in range(B)], axis=0
    )
    return full.astype(np.float32)


def run_sharded(inputs, trace=False, t_len=T, mm_dtype=MM_DTYPE):
    nc = _get_nc(t_len, mm_dtype)
    in_maps = make_in_maps(t_len=t_len, mm_dtype=mm_dtype, **inputs)
    res = run_bass_kernel_spmd(nc, in_maps, core_ids=list(range(N_CORES)),
                               trace=trace)
    return gather_output(res.results, t_len), res


def kernel(x, W_qkv, W_q_lsr, W_k_lsr, lsr_core, W_o):
    out, _ = run_sharded(dict(x=x, W_qkv=W_qkv, W_q_lsr=W_q_lsr,
                              W_k_lsr=W_k_lsr, lsr_core=lsr_core, W_o=W_o))
    return out


# revision 11
# speedup vs baseline: 2.9029x; 1.0455x over previous
"""Multi-head self-attention with low-rank bilinear scores (LSR) on 8 TRN2 cores.

Sharding: core i handles batch b = i//4 and 4 heads (i%4)*4..+4.
Per core, everything runs in the "transposed" orientation:
  qkv^T = W^T @ x^T   (contract over D, x pre-transposed on host)
  ql^T / kl^T via block-diagonal packed LSR weights (2 heads per matmul)
  scores^T [s, t] = kl^T-slices @ ql^T  (K = RANK = 32, per-head 32-row strips)
  P = exp(scores^T) * causal_mask;  PV uses V with an appended ones column so
  the softmax denominator falls out of the same matmul (row 64 of y_psum).
  y^T gets normalized by 1/den (broadcast via a K=1 ones matmul), then the
  W_o row-slice projection produces a per-core partial [T, D] output.
Host: per-batch sum over the 4 cores sharing that batch (W_o is row-sharded),
then stack the two batches.
"""

import numpy as np
from contextlib import ExitStack

import concourse.tile as tile
from concourse import bacc, mybir
from concourse.bass_utils import run_bass_kernel_spmd

B, T, D = 2, 2048, 1024
H, RANK = 16, 32
DH = D // H  # 64
N_CORES = 8
HPC = 4  # heads per core
CPB = N_CORES // B  # cores per batch
W = 512  # t-window (= one fp32 PSUM bank)
KC = D // 128  # k-chunks over D
F32 = mybir.dt.float32
BF16 = mybir.dt.bfloat16
F32R = mybir.dt.float32r

MM_DTYPE = BF16  # dtype of all TensorE-facing operands (except the tiny
                 # fp32 denominator-broadcast matmul)

_NC_CACHE = {}


def _build_nc(t_len=T, mm_dtype=MM_DTYPE):
    """Build + compile the single-core SPMD program (same NEFF on all cores)."""
    NW = t_len // W  # windows
    SBW = W // 128  # s-chunks per window
    NSB = t_len // 128  # total s-chunks
    DT = mm_dtype

    nc = bacc.Bacc("TRN2", target_bir_lowering=False, debug=False,
                   num_devices=N_CORES)

    xt_d = nc.dram_tensor("xt", [D, t_len], DT, kind="ExternalInput")
    wqk_d = nc.dram_tensor("wqk", [128, KC, 2 * HPC * DH], DT, kind="ExternalInput")
    wv_d = nc.dram_tensor("wv", [128, KC, HPC * DH], DT, kind="ExternalInput")
    wql_d = nc.dram_tensor("wql", [128, HPC // 2, 2 * RANK], DT, kind="ExternalInput")
    wkl_d = nc.dram_tensor("wkl", [128, HPC // 2, 2 * RANK], DT, kind="ExternalInput")
    wo_d = nc.dram_tensor("wo", [128, 2, D], DT, kind="ExternalInput")
    mask_d = nc.dram_tensor("masks", [128, SBW, W], DT, kind="ExternalInput")
    out_d = nc.dram_tensor("out", [t_len, D], F32, kind="ExternalOutput")

    with tile.TileContext(nc) as tc, ExitStack() as ctx:
        const = ctx.enter_context(tc.tile_pool(name="const", bufs=1))
        xt_pool = ctx.enter_context(tc.tile_pool(name="xtp", bufs=2))
        qkt_pool = ctx.enter_context(tc.tile_pool(name="qktp", bufs=2))
        klt_pool = ctx.enter_context(tc.tile_pool(name="kltp", bufs=NW))
        qlt_pool = ctx.enter_context(tc.tile_pool(name="qltp", bufs=2))
        v_pool = ctx.enter_context(tc.tile_pool(name="vp", bufs=NSB))
        p_pool = ctx.enter_context(tc.tile_pool(name="pp", bufs=3))
        y_pool = ctx.enter_context(tc.tile_pool(name="yp", bufs=2))
        yun_pool = ctx.enter_context(tc.tile_pool(name="yunp", bufs=2 * HPC))
        dr_pool = ctx.enter_context(tc.tile_pool(name="drp", bufs=4))
        stage_pool = ctx.enter_context(tc.tile_pool(name="stgp", bufs=2))
        acc = ctx.enter_context(tc.tile_pool(name="accp", bufs=4, space="PSUM"))
        stp = ctx.enter_context(tc.tile_pool(name="stpp", bufs=2, space="PSUM"))

        wqk_sb = const.tile([128, KC, 2 * HPC * DH], DT)
        nc.sync.dma_start(wqk_sb[:], wqk_d.ap()[:, :, :])
        wv_sb = const.tile([128, KC, HPC * DH], DT)
        nc.sync.dma_start(wv_sb[:], wv_d.ap()[:, :, :])
        wql_sb = const.tile([128, HPC // 2, 2 * RANK], DT)
        nc.sync.dma_start(wql_sb[:], wql_d.ap()[:, :, :])
        wkl_sb = const.tile([128, HPC // 2, 2 * RANK], DT)
        nc.sync.dma_start(wkl_sb[:], wkl_d.ap()[:, :, :])
        wo_sb = const.tile([128, 2, D], DT)
        nc.sync.dma_start(wo_sb[:], wo_d.ap()[:, :, :])
        mask_sb = const.tile([128, SBW, W], DT)
        nc.sync.dma_start(mask_sb[:], mask_d.ap()[:, :, :])
        ones_sb = const.tile([1, DH], F32)
        nc.vector.memset(ones_sb[:], 1.0)

        xt_dram = xt_d.ap().rearrange("(kc p) t -> p kc t", p=128)

        klt_tiles = [None] * NW
        qlt_tiles = [None] * NW
        v_tiles = [None] * NSB

        # ------------------------------------------------------------------
        # Emission is a "zipper": the attention pair-chain of window w (ACT
        # exp-bound, ~60% PE duty on its own) is interleaved with filler PE
        # work — the QKV/LSR matmul groups of window w+1 and the norm+W_o
        # groups of window w-1 — so the PE instruction stream never starves
        # and the HAM clock gate stays at 8/8.
        # ------------------------------------------------------------------

        def qkv_lsr_closures(w):
            """Filler closures producing qt/kt/v/qlt/klt for window w."""
            xt_w = xt_pool.tile([128, KC, W], DT, tag="xt", name=f"xt_{w}")
            nc.sync.dma_start(xt_w[:], xt_dram[:, :, w * W:(w + 1) * W])
            qt_w = qkt_pool.tile([128, 2, W], DT, tag="qt", name=f"qt_{w}")
            kt_w = qkt_pool.tile([128, 2, W], DT, tag="kt", name=f"kt_{w}")
            qlt_w = qlt_pool.tile([128, W], DT, tag="qlt", name=f"qlt_{w}")
            klt_w = klt_pool.tile([128, W], DT, tag="klt", name=f"klt_{w}")
            qlt_tiles[w] = qlt_w
            klt_tiles[w] = klt_w
            cls = []

            def qk_group(pc, qt_w=qt_w, kt_w=kt_w, xt_w=xt_w):
                ps = acc.tile([128, W], F32, tag="acc")
                for kc in range(KC):
                    nc.tensor.matmul(
                        ps[:],
                        wqk_sb[:, kc, pc * 128:(pc + 1) * 128],
                        xt_w[:, kc, :],
                        start=(kc == 0), stop=(kc == KC - 1),
                    )
                dst = (qt_w if pc < 2 else kt_w)[:, pc % 2, :]
                nc.vector.tensor_copy(dst, ps[:])

            def v_group(tl, xt_w=xt_w, w=w):
                sb = w * SBW + tl
                ps = acc.tile([128, HPC * DH], F32, tag="acc")
                for kc in range(KC):
                    nc.tensor.matmul(
                        ps[:],
                        xt_w[:, kc, tl * 128:(tl + 1) * 128],
                        wv_sb[:, kc, :],
                        start=(kc == 0), stop=(kc == KC - 1),
                    )
                vt = v_pool.tile([128, HPC, DH + 1], DT, tag="v")
                nc.vector.tensor_copy(
                    vt[:, :, 0:DH], ps[:].rearrange("p (h d) -> p h d", h=HPC)
                )
                nc.vector.memset(vt[:, :, DH:DH + 1], 1.0)
                v_tiles[sb] = vt

            def lsr_group(pair, qt_w=qt_w, kt_w=kt_w, qlt_w=qlt_w, klt_w=klt_w):
                ps = acc.tile([64, W], F32, tag="acc")
                nc.tensor.matmul(ps[:], wql_sb[:, pair, :], qt_w[:, pair, :],
                                 start=True, stop=True)
                nc.vector.tensor_copy(qlt_w[pair * 64:(pair + 1) * 64, :], ps[:])
                ps = acc.tile([64, W], F32, tag="acc")
                nc.tensor.matmul(ps[:], wkl_sb[:, pair, :], kt_w[:, pair, :],
                                 start=True, stop=True)
                nc.vector.tensor_copy(klt_w[pair * 64:(pair + 1) * 64, :], ps[:])

            for pc in range(4):
                cls.append(lambda pc=pc: qk_group(pc))
            for tl in range(SBW):
                cls.append(lambda tl=tl: v_group(tl))
            for pair in range(2):
                cls.append(lambda pair=pair: lsr_group(pair))
            return cls

        def norm_wo_closures(w, yun_tiles):
            """Filler closures: normalize window w's y^T, then W_o groups."""
            y_w = y_pool.tile([128, 2, W], DT, tag="y", name=f"y_{w}")
            cls = []

            def norm_head(h, y_w=y_w, yun_tiles=yun_tiles):
                yun = yun_tiles[h]
                rec = dr_pool.tile([1, W], F32, tag="rec")
                nc.vector.reciprocal(rec[:], yun[DH:DH + 1, :])
                bc = acc.tile([64, W], F32, tag="acc")
                nc.tensor.matmul(bc[:], ones_sb[0:1, :], rec[0:1, :],
                                 start=True, stop=True)
                bc_sb = dr_pool.tile([64, W], F32, tag="bcs")
                nc.vector.tensor_copy(bc_sb[:], bc[:])
                nc.vector.tensor_mul(
                    y_w[(h % 2) * 64:(h % 2) * 64 + 64, h // 2, :],
                    yun[0:DH, :], bc_sb[:],
                )

            def wo_group(tl, y_w=y_w, w=w):
                stg = stage_pool.tile([128, D], F32, tag="stg")
                for nch in range(D // W):
                    ps = acc.tile([128, W], F32, tag="acc")
                    for kc2 in range(2):
                        nc.tensor.matmul(
                            ps[:],
                            y_w[:, kc2, tl * 128:(tl + 1) * 128],
                            wo_sb[:, kc2, nch * W:(nch + 1) * W],
                            start=(kc2 == 0), stop=(kc2 == 1),
                        )
                    nc.vector.tensor_copy(stg[:, nch * W:(nch + 1) * W], ps[:])
                r0 = w * W + tl * 128
                nc.sync.dma_start(out_d.ap()[r0:r0 + 128, :], stg[:])

            for h in range(HPC):
                cls.append(lambda h=h: norm_head(h))
            for tl in range(SBW):
                cls.append(lambda tl=tl: wo_group(tl))
            return cls

        def attention_pairs(w):
            """Pair closures for window w (all heads), plus per-head state."""
            n_sb = (w + 1) * SBW
            npairs = n_sb // 2
            qlt_w = qlt_tiles[w]
            yun_tiles = []
            state = {}
            cls = []

            def st_exp(h, i, qlt_w=qlt_w, w=w):
                sp = stp.tile([128, 2, W], F32, tag="st")
                for j in (0, 1):
                    sb = 2 * i + j
                    nc.tensor.matmul(
                        sp[:, j, :],
                        klt_tiles[sb // SBW][32 * h:32 * h + 32,
                                             (sb % SBW) * 128:(sb % SBW + 1) * 128],
                        qlt_w[32 * h:32 * h + 32, :],
                        start=True, stop=True,
                        tile_position=(32 * h, 0),
                    )
                pt = p_pool.tile([128, 2, W], DT, tag="pt")
                nc.scalar.activation(pt[:], sp[:],
                                     mybir.ActivationFunctionType.Exp)
                if (2 * i) // SBW == w:  # diagonal window -> causal mask
                    m = (2 * i) % SBW
                    nc.vector.tensor_mul(pt[:], pt[:], mask_sb[:, m:m + 2, :])
                state[(h, i)] = pt

            def pv(h, i, n_sb=n_sb, npairs=npairs, yun_tiles=yun_tiles):
                if i == 0:
                    state[("yps", h)] = acc.tile([DH + 1, W], F32, tag="acc",
                                                 name=f"yps_{h}")
                yps = state[("yps", h)]
                pt = state.pop((h, i))
                for j in (0, 1):
                    sb = 2 * i + j
                    nc.tensor.matmul(
                        yps[:], v_tiles[sb][:, h, :], pt[:, j, :],
                        start=(sb == 0), stop=(sb == n_sb - 1),
                    )
                if i == npairs - 1:
                    yun = yun_pool.tile([DH + 1, W], F32, tag="yun")
                    nc.vector.tensor_copy(yun[:], yps[:])
                    yun_tiles.append(yun)
                    state.pop(("yps", h))

            for h in range(HPC):
                cls.append(lambda h=h: st_exp(h, 0))
                for i in range(npairs):
                    if i + 1 < npairs:
                        cls.append(lambda h=h, i=i: st_exp(h, i + 1))
                    cls.append(lambda h=h, i=i: pv(h, i))
            return cls, yun_tiles

        def zip_emit(pairs, fillers):
            """Emit pair closures with filler closures spread between them."""
            nf, np_ = len(fillers), max(1, len(pairs))
            fi = 0
            for k, p in enumerate(pairs):
                p()
                target = (k + 1) * nf // np_
                while fi < target:
                    fillers[fi]()
                    fi += 1
            while fi < nf:
                fillers[fi]()
                fi += 1

        # startup: window 0 inputs
        for c in qkv_lsr_closures(0):
            c()

        pending = None  # (w, yun_tiles) awaiting norm+Wo
        for w in range(NW):
            pairs, yun_tiles = attention_pairs(w)
            fillers = []
            if w + 1 < NW:
                fillers += qkv_lsr_closures(w + 1)
            if pending is not None:
                fillers += norm_wo_closures(*pending)
            zip_emit(pairs, fillers)
            pending = (w, yun_tiles)

        for c in norm_wo_closures(*pending):
            c()

    nc.compile()
    return nc


def _get_nc(t_len=T, mm_dtype=MM_DTYPE):
    key = (t_len, mm_dtype)
    if key not in _NC_CACHE:
        _NC_CACHE[key] = _build_nc(t_len, mm_dtype)
    return _NC_CACHE[key]


def _np_dt(mm_dtype):
    return mybir.dt.np(mm_dtype)


def _build_masks(t_len=T):
    SBW = W // 128
    p = np.arange(128)[:, None]
    c = np.arange(W)[None, :]
    masks = np.zeros((128, SBW, W), np.float32)
    for m in range(SBW):
        masks[:, m, :] = (c - p >= 128 * m).astype(np.float32)
    return masks


def make_in_maps(x, W_qkv, W_q_lsr, W_k_lsr, lsr_core, W_o, t_len=T,
                 mm_dtype=MM_DTYPE):
    x = np.asarray(x, np.float32)
    W_qkv = np.asarray(W_qkv, np.float32)
    W_q_lsr = np.asarray(W_q_lsr, np.float32)
    W_k_lsr = np.asarray(W_k_lsr, np.float32)
    lsr_core = np.asarray(lsr_core, np.float32)
    W_o = np.asarray(W_o, np.float32)
    ndt = _np_dt(mm_dtype)

    d = x.shape[-1]
    masks = _build_masks(t_len).astype(ndt)
    scale = lsr_core / np.sqrt(np.float32(RANK))  # [H, R]

    in_maps = []
    for i in range(N_CORES):
        b = i // CPB
        hg = i % CPB
        h0 = hg * HPC
        col0 = h0 * DH
        wq = W_qkv[:, col0:col0 + HPC * DH]
        wk = W_qkv[:, d + col0:d + col0 + HPC * DH]
        wv = W_qkv[:, 2 * d + col0:2 * d + col0 + HPC * DH]
        wqk = np.concatenate([wq, wk], axis=1).reshape(KC, 128, 2 * HPC * DH)
        wqk = wqk.transpose(1, 0, 2)
        wv_p = wv.reshape(KC, 128, HPC * DH).transpose(1, 0, 2)
        wql = np.zeros((HPC // 2, 128, 2 * RANK), np.float32)
        wkl = np.zeros((HPC // 2, 128, 2 * RANK), np.float32)
        for pair in range(HPC // 2):
            for k in range(2):
                h = h0 + pair * 2 + k
                wql[pair, k * DH:(k + 1) * DH, k * RANK:(k + 1) * RANK] = (
                    W_q_lsr[h] * scale[h][None, :]
                )
                wkl[pair, k * DH:(k + 1) * DH, k * RANK:(k + 1) * RANK] = W_k_lsr[h]
        wo = W_o[col0:col0 + HPC * DH, :].reshape(2, 128, d)
        in_maps.append({
            "xt": np.ascontiguousarray(x[b, :t_len].T).astype(ndt),
            "wqk": np.ascontiguousarray(wqk).astype(ndt),
            "wv": np.ascontiguousarray(wv_p).astype(ndt),
            "wql": np.ascontiguousarray(wql.transpose(1, 0, 2)).astype(ndt),
            "wkl": np.ascontiguousarray(wkl.transpose(1, 0, 2)).astype(ndt),
            "wo": np.ascontiguousarray(wo.transpose(1, 0, 2)).astype(ndt),
            "masks": masks,
        })
    return in_maps


def gather_output(results, t_len=T):
    outs = [np.asarray(results[i]["out"], np.float32) for i in range(N_CORES)]
    full = np.stack(
        [sum(outs[b * CPB:(b + 1) * CPB]) for b in range(B)], axis=0
    )
    return full.astype(np.float32)


def run_sharded(inputs, trace=False, t_len=T, mm_dtype=MM_DTYPE):
    nc = _get_nc(t_len, mm_dtype)
    in_maps = make_in_maps(t_len=t_len, mm_dtype=mm_dtype, **inputs)
    res = run_bass_kernel_spmd(nc, in_maps, core_ids=list(range(N_CORES)),
                               trace=trace)
    return gather_output(res.results, t_len), res


def kernel(x, W_qkv, W_q_lsr, W_k_lsr, lsr_core, W_o):
    out, _ = run_sharded(dict(x=x, W_qkv=W_qkv, W_q_lsr=W_q_lsr,
                              W_k_lsr=W_k_lsr, lsr_core=lsr_core, W_o=W_o))
    return out


# revision 13
# speedup vs baseline: 3.2432x; 1.1172x over previous
"""Multi-head self-attention with low-rank bilinear scores (LSR) on 8 TRN2 cores.

Sharding: core i handles batch b = i//4 and 4 heads (i%4)*4..+4.
Per core, everything runs in the "transposed" orientation:
  qkv^T = W^T @ x^T   (contract over D, x pre-transposed on host)
  ql^T / kl^T via block-diagonal packed LSR weights (2 heads per matmul)
  scores^T [s, t] = kl^T-slices @ ql^T  (K = RANK = 32, per-head 32-row strips)
  P = exp(scores^T) * causal_mask;  PV uses V with an appended ones column so
  the softmax denominator falls out of the same matmul (row 64 of y_psum).
  y^T gets normalized by 1/den (broadcast via a K=1 ones matmul), then the
  W_o row-slice projection produces a per-core partial [T, D] output.
Host: per-batch sum over the 4 cores sharing that batch (W_o is row-sharded),
then stack the two batches.
"""

import numpy as np
from contextlib import ExitStack

import concourse.tile as tile
from concourse import bacc, mybir
from concourse.bass_utils import run_bass_kernel_spmd

B, T, D = 2, 2048, 1024
H, RANK = 16, 32
DH = D // H  # 64
N_CORES = 8
HPC = 4  # heads per core
CPB = N_CORES // B  # cores per batch
W = 512  # t-window (= one fp32 PSUM bank)
KC = D // 128  # k-chunks over D
F32 = mybir.dt.float32
BF16 = mybir.dt.bfloat16
F32R = mybir.dt.float32r

MM_DTYPE = BF16  # dtype of all TensorE-facing operands (except the tiny
                 # fp32 denominator-broadcast matmul)

_NC_CACHE = {}


def _build_nc(t_len=T, mm_dtype=MM_DTYPE):
    """Build + compile the single-core SPMD program (same NEFF on all cores)."""
    NW = t_len // W  # windows
    SBW = W // 128  # s-chunks per window
    NSB = t_len // 128  # total s-chunks
    DT = mm_dtype

    nc = bacc.Bacc("TRN2", target_bir_lowering=False, debug=False,
                   num_devices=N_CORES)

    xt_d = nc.dram_tensor("xt", [D, t_len], DT, kind="ExternalInput")
    wqk_d = nc.dram_tensor("wqk", [128, KC, 2 * HPC * DH], DT, kind="ExternalInput")
    wv_d = nc.dram_tensor("wv", [128, KC, HPC * DH], DT, kind="ExternalInput")
    wql_d = nc.dram_tensor("wql", [128, HPC // 2, 2 * RANK], DT, kind="ExternalInput")
    wkl_d = nc.dram_tensor("wkl", [128, HPC // 2, 2 * RANK], DT, kind="ExternalInput")
    wo_d = nc.dram_tensor("wo", [128, 2, D], DT, kind="ExternalInput")
    mask_d = nc.dram_tensor("masks", [128, SBW, W], DT, kind="ExternalInput")
    out_d = nc.dram_tensor("out", [t_len, D], F32, kind="ExternalOutput")

    with tile.TileContext(nc) as tc, ExitStack() as ctx:
        const = ctx.enter_context(tc.tile_pool(name="const", bufs=1))
        xt_pool = ctx.enter_context(tc.tile_pool(name="xtp", bufs=2))
        qkt_pool = ctx.enter_context(tc.tile_pool(name="qktp", bufs=2))
        klt_pool = ctx.enter_context(tc.tile_pool(name="kltp", bufs=NW))
        qlt_pool = ctx.enter_context(tc.tile_pool(name="qltp", bufs=2))
        v_pool = ctx.enter_context(tc.tile_pool(name="vp", bufs=NSB))
        p_pool = ctx.enter_context(tc.tile_pool(name="pp", bufs=3))
        y_pool = ctx.enter_context(tc.tile_pool(name="yp", bufs=2))
        yun_pool = ctx.enter_context(tc.tile_pool(name="yunp", bufs=2 * HPC))
        dr_pool = ctx.enter_context(tc.tile_pool(name="drp", bufs=4))
        stage_pool = ctx.enter_context(tc.tile_pool(name="stgp", bufs=2))
        dram_pool = ctx.enter_context(tc.tile_pool(name="dramp", bufs=4,
                                                   space="DRAM"))
        acc = ctx.enter_context(tc.tile_pool(name="accp", bufs=4, space="PSUM"))
        stp = ctx.enter_context(tc.tile_pool(name="stpp", bufs=2, space="PSUM"))

        wqk_sb = const.tile([128, KC, 2 * HPC * DH], DT)
        nc.sync.dma_start(wqk_sb[:], wqk_d.ap()[:, :, :])
        wv_sb = const.tile([128, KC, HPC * DH], DT)
        nc.sync.dma_start(wv_sb[:], wv_d.ap()[:, :, :])
        wql_sb = const.tile([128, HPC // 2, 2 * RANK], DT)
        nc.sync.dma_start(wql_sb[:], wql_d.ap()[:, :, :])
        wkl_sb = const.tile([128, HPC // 2, 2 * RANK], DT)
        nc.sync.dma_start(wkl_sb[:], wkl_d.ap()[:, :, :])
        wo_sb = const.tile([128, 2, D], DT)
        nc.sync.dma_start(wo_sb[:], wo_d.ap()[:, :, :])
        mask_sb = const.tile([128, SBW, W], DT)
        nc.sync.dma_start(mask_sb[:], mask_d.ap()[:, :, :])
        ones_sb = const.tile([1, DH], F32)
        nc.vector.memset(ones_sb[:], 1.0)

        xt_dram = xt_d.ap().rearrange("(kc p) t -> p kc t", p=128)

        klt_tiles = [None] * NW
        qlt_tiles = [None] * NW
        v_tiles = [None] * NSB

        # ------------------------------------------------------------------
        # Emission is a "zipper": the attention pair-chain of window w (ACT
        # exp-bound, ~60% PE duty on its own) is interleaved with filler PE
        # work — the QKV/LSR matmul groups of window w+1 and the norm+W_o
        # groups of window w-1 — so the PE instruction stream never starves
        # and the HAM clock gate stays at 8/8.
        # ------------------------------------------------------------------

        def qkv_lsr_closures(w):
            """Filler closures producing qt/kt/v/qlt/klt for window w."""
            xt_w = xt_pool.tile([128, KC, W], DT, tag="xt", name=f"xt_{w}")
            nc.sync.dma_start(xt_w[:], xt_dram[:, :, w * W:(w + 1) * W])
            qt_w = qkt_pool.tile([128, 2, W], DT, tag="qt", name=f"qt_{w}")
            kt_w = qkt_pool.tile([128, 2, W], DT, tag="kt", name=f"kt_{w}")
            qlt_w = qlt_pool.tile([128, W], DT, tag="qlt", name=f"qlt_{w}")
            klt_w = klt_pool.tile([128, W], DT, tag="klt", name=f"klt_{w}")
            qlt_tiles[w] = qlt_w
            klt_tiles[w] = klt_w
            cls = []

            def qk_group(pc, qt_w=qt_w, kt_w=kt_w, xt_w=xt_w):
                ps = acc.tile([128, W], F32, tag="acc")
                for kc in range(KC):
                    nc.tensor.matmul(
                        ps[:],
                        wqk_sb[:, kc, pc * 128:(pc + 1) * 128],
                        xt_w[:, kc, :],
                        start=(kc == 0), stop=(kc == KC - 1),
                    )
                dst = (qt_w if pc < 2 else kt_w)[:, pc % 2, :]
                nc.vector.tensor_copy(dst, ps[:])

            def v_group(tl, xt_w=xt_w, w=w):
                sb = w * SBW + tl
                ps = acc.tile([128, HPC * DH], F32, tag="acc")
                for kc in range(KC):
                    nc.tensor.matmul(
                        ps[:],
                        xt_w[:, kc, tl * 128:(tl + 1) * 128],
                        wv_sb[:, kc, :],
                        start=(kc == 0), stop=(kc == KC - 1),
                    )
                vt = v_pool.tile([128, HPC, DH + 1], DT, tag="v")
                nc.vector.tensor_copy(
                    vt[:, :, 0:DH], ps[:].rearrange("p (h d) -> p h d", h=HPC)
                )
                nc.vector.memset(vt[:, :, DH:DH + 1], 1.0)
                v_tiles[sb] = vt

            def lsr_group(pair, qt_w=qt_w, kt_w=kt_w, qlt_w=qlt_w, klt_w=klt_w):
                ps = acc.tile([64, W], F32, tag="acc")
                nc.tensor.matmul(ps[:], wql_sb[:, pair, :], qt_w[:, pair, :],
                                 start=True, stop=True)
                nc.vector.tensor_copy(qlt_w[pair * 64:(pair + 1) * 64, :], ps[:])
                ps = acc.tile([64, W], F32, tag="acc")
                nc.tensor.matmul(ps[:], wkl_sb[:, pair, :], kt_w[:, pair, :],
                                 start=True, stop=True)
                nc.vector.tensor_copy(klt_w[pair * 64:(pair + 1) * 64, :], ps[:])

            for pc in range(4):
                cls.append(lambda pc=pc: qk_group(pc))
            for tl in range(SBW):
                cls.append(lambda tl=tl: v_group(tl))
            for pair in range(2):
                cls.append(lambda pair=pair: lsr_group(pair))
            return cls

        def norm_wo_closures(w, yun_tiles):
            """Filler closures: normalize window w's y^T, then W_o groups."""
            y_w = y_pool.tile([128, 2, W], DT, tag="y", name=f"y_{w}")
            cls = []

            def norm_head(h, y_w=y_w, yun_tiles=yun_tiles):
                yun = yun_tiles[h]
                rec = dr_pool.tile([1, W], F32, tag="rec")
                nc.vector.reciprocal(rec[:], yun[DH:DH + 1, :])
                # broadcast 1/den across 64 partitions via a DRAM bounce +
                # stride-0-partition DMA instead of a K=1 fp32 matmul: keeps
                # the whole broadcast off the PE stream.
                rec_d = dram_pool.tile([1, W], F32, tag="recd")
                nc.sync.dma_start(rec_d[:], rec[:])
                bc_sb = dr_pool.tile([64, W], F32, tag="bcs")
                nc.sync.dma_start(bc_sb[:], rec_d[:].to_broadcast([64, W]))
                nc.vector.tensor_mul(
                    y_w[(h % 2) * 64:(h % 2) * 64 + 64, h // 2, :],
                    yun[0:DH, :], bc_sb[:],
                )

            def wo_group(tl, y_w=y_w, w=w):
                stg = stage_pool.tile([128, D], F32, tag="stg")
                for nch in range(D // W):
                    ps = acc.tile([128, W], F32, tag="acc")
                    for kc2 in range(2):
                        nc.tensor.matmul(
                            ps[:],
                            y_w[:, kc2, tl * 128:(tl + 1) * 128],
                            wo_sb[:, kc2, nch * W:(nch + 1) * W],
                            start=(kc2 == 0), stop=(kc2 == 1),
                        )
                    nc.vector.tensor_copy(stg[:, nch * W:(nch + 1) * W], ps[:])
                r0 = w * W + tl * 128
                nc.sync.dma_start(out_d.ap()[r0:r0 + 128, :], stg[:])

            for h in range(HPC):
                cls.append(lambda h=h: norm_head(h))
            for tl in range(SBW):
                cls.append(lambda tl=tl: wo_group(tl))
            return cls

        def attention_pairs(w):
            """Pair closures for window w (all heads), plus per-head state."""
            n_sb = (w + 1) * SBW
            npairs = n_sb // 2
            qlt_w = qlt_tiles[w]
            yun_tiles = []
            state = {}
            cls = []

            def st_exp(h, i, qlt_w=qlt_w, w=w):
                sp = stp.tile([128, 2, W], F32, tag="st")
                for j in (0, 1):
                    sb = 2 * i + j
                    nc.tensor.matmul(
                        sp[:, j, :],
                        klt_tiles[sb // SBW][32 * h:32 * h + 32,
                                             (sb % SBW) * 128:(sb % SBW + 1) * 128],
                        qlt_w[32 * h:32 * h + 32, :],
                        start=True, stop=True,
                        tile_position=(32 * h, 0),
                    )
                pt = p_pool.tile([128, 2, W], DT, tag="pt")
                nc.scalar.activation(pt[:], sp[:],
                                     mybir.ActivationFunctionType.Exp)
                if (2 * i) // SBW == w:  # diagonal window -> causal mask
                    m = (2 * i) % SBW
                    nc.vector.tensor_mul(pt[:], pt[:], mask_sb[:, m:m + 2, :])
                state[(h, i)] = pt

            def pv(h, i, n_sb=n_sb, npairs=npairs, yun_tiles=yun_tiles):
                if i == 0:
                    state[("yps", h)] = acc.tile([DH + 1, W], F32, tag="acc",
                                                 name=f"yps_{h}")
                yps = state[("yps", h)]
                pt = state.pop((h, i))
                for j in (0, 1):
                    sb = 2 * i + j
                    nc.tensor.matmul(
                        yps[:], v_tiles[sb][:, h, :], pt[:, j, :],
                        start=(sb == 0), stop=(sb == n_sb - 1),
                    )
                if i == npairs - 1:
                    yun = yun_pool.tile([DH + 1, W], F32, tag="yun")
                    nc.vector.tensor_copy(yun[:], yps[:])
                    yun_tiles.append(yun)
                    state.pop(("yps", h))

            for h in range(HPC):
                cls.append(lambda h=h: st_exp(h, 0))
                for i in range(npairs):
                    if i + 1 < npairs:
                        cls.append(lambda h=h, i=i: st_exp(h, i + 1))
                    cls.append(lambda h=h, i=i: pv(h, i))
            return cls, yun_tiles

        def zip_emit(pairs, fillers):
            """Emit pair closures with filler closures spread between them."""
            nf, np_ = len(fillers), max(1, len(pairs))
            fi = 0
            for k, p in enumerate(pairs):
                p()
                target = (k + 1) * nf // np_
                while fi < target:
                    fillers[fi]()
                    fi += 1
            while fi < nf:
                fillers[fi]()
                fi += 1

        # startup: window 0 inputs
        for c in qkv_lsr_closures(0):
            c()

        pending = None  # (w, yun_tiles) awaiting norm+Wo
        for w in range(NW):
            pairs, yun_tiles = attention_pairs(w)
            fillers = []
            if w + 1 < NW:
                fillers += qkv_lsr_closures(w + 1)
            if pending is not None:
                fillers += norm_wo_closures(*pending)
            zip_emit(pairs, fillers)
            pending = (w, yun_tiles)

        for c in norm_wo_closures(*pending):
            c()

    nc.compile()
    return nc


def _get_nc(t_len=T, mm_dtype=MM_DTYPE):
    key = (t_len, mm_dtype)
    if key not in _NC_CACHE:
        _NC_CACHE[key] = _build_nc(t_len, mm_dtype)
    return _NC_CACHE[key]


def _np_dt(mm_dtype):
    return mybir.dt.np(mm_dtype)


def _build_masks(t_len=T):
    SBW = W // 128
    p = np.arange(128)[:, None]
    c = np.arange(W)[None, :]
    masks = np.zeros((128, SBW, W), np.float32)
    for m in range(SBW):
        masks[:, m, :] = (c - p >= 128 * m).astype(np.float32)
    return masks


def make_in_maps(x, W_qkv, W_q_lsr, W_k_lsr, lsr_core, W_o, t_len=T,
                 mm_dtype=MM_DTYPE):
    x = np.asarray(x, np.float32)
    W_qkv = np.asarray(W_qkv, np.float32)
    W_q_lsr = np.asarray(W_q_lsr, np.float32)
    W_k_lsr = np.asarray(W_k_lsr, np.float32)
    lsr_core = np.asarray(lsr_core, np.float32)
    W_o = np.asarray(W_o, np.float32)
    ndt = _np_dt(mm_dtype)

    d = x.shape[-1]
    masks = _build_masks(t_len).astype(ndt)
    scale = lsr_core / np.sqrt(np.float32(RANK))  # [H, R]

    in_maps = []
    for i in range(N_CORES):
        b = i // CPB
        hg = i % CPB
        h0 = hg * HPC
        col0 = h0 * DH
        wq = W_qkv[:, col0:col0 + HPC * DH]
        wk = W_qkv[:, d + col0:d + col0 + HPC * DH]
        wv = W_qkv[:, 2 * d + col0:2 * d + col0 + HPC * DH]
        wqk = np.concatenate([wq, wk], axis=1).reshape(KC, 128, 2 * HPC * DH)
        wqk = wqk.transpose(1, 0, 2)
        wv_p = wv.reshape(KC, 128, HPC * DH).transpose(1, 0, 2)
        wql = np.zeros((HPC // 2, 128, 2 * RANK), np.float32)
        wkl = np.zeros((HPC // 2, 128, 2 * RANK), np.float32)
        for pair in range(HPC // 2):
            for k in range(2):
                h = h0 + pair * 2 + k
                wql[pair, k * DH:(k + 1) * DH, k * RANK:(k + 1) * RANK] = (
                    W_q_lsr[h] * scale[h][None, :]
                )
                wkl[pair, k * DH:(k + 1) * DH, k * RANK:(k + 1) * RANK] = W_k_lsr[h]
        wo = W_o[col0:col0 + HPC * DH, :].reshape(2, 128, d)
        in_maps.append({
            "xt": np.ascontiguousarray(x[b, :t_len].T).astype(ndt),
            "wqk": np.ascontiguousarray(wqk).astype(ndt),
            "wv": np.ascontiguousarray(wv_p).astype(ndt),
            "wql": np.ascontiguousarray(wql.transpose(1, 0, 2)).astype(ndt),
            "wkl": np.ascontiguousarray(wkl.transpose(1, 0, 2)).astype(ndt),
            "wo": np.ascontiguousarray(wo.transpose(1, 0, 2)).astype(ndt),
            "masks": masks,
        })
    return in_maps


def gather_output(results, t_len=T):
    outs = [np.asarray(results[i]["out"], np.float32) for i in range(N_CORES)]
    full = np.stack(
        [sum(outs[b * CPB:(b + 1) * CPB]) for b in range(B)], axis=0
    )
    return full.astype(np.float32)


def run_sharded(inputs, trace=False, t_len=T, mm_dtype=MM_DTYPE):
    nc = _get_nc(t_len, mm_dtype)
    in_maps = make_in_maps(t_len=t_len, mm_dtype=mm_dtype, **inputs)
    res = run_bass_kernel_spmd(nc, in_maps, core_ids=list(range(N_CORES)),
                               trace=trace)
    return gather_output(res.results, t_len), res


def kernel(x, W_qkv, W_q_lsr, W_k_lsr, lsr_core, W_o):
    out, _ = run_sharded(dict(x=x, W_qkv=W_qkv, W_q_lsr=W_q_lsr,
                              W_k_lsr=W_k_lsr, lsr_core=lsr_core, W_o=W_o))
    return out


# revision 14
# speedup vs baseline: 3.5447x; 1.0930x over previous
"""Multi-head self-attention with low-rank bilinear scores (LSR) on 8 TRN2 cores.

Sharding: core i handles batch b = i//4 and 4 heads (i%4)*4..+4.
Per core, everything runs in the "transposed" orientation:
  qkv^T = W^T @ x^T   (contract over D, x pre-transposed on host)
  ql^T / kl^T via block-diagonal packed LSR weights (2 heads per matmul)
  scores^T [s, t] = kl^T-slices @ ql^T  (K = RANK = 32, per-head 32-row strips)
  P = exp(scores^T) * causal_mask;  PV uses V with an appended ones column so
  the softmax denominator falls out of the same matmul (row 64 of y_psum).
  y^T gets normalized by 1/den (broadcast via a K=1 ones matmul), then the
  W_o row-slice projection produces a per-core partial [T, D] output.
Host: per-batch sum over the 4 cores sharing that batch (W_o is row-sharded),
then stack the two batches.
"""

import numpy as np
from contextlib import ExitStack

import concourse.tile as tile
from concourse import bacc, mybir
from concourse.bass_utils import run_bass_kernel_spmd

B, T, D = 2, 2048, 1024
H, RANK = 16, 32
DH = D // H  # 64
N_CORES = 8
HPC = 4  # heads per core
CPB = N_CORES // B  # cores per batch
W = 512  # t-window (= one fp32 PSUM bank)
KC = D // 128  # k-chunks over D
F32 = mybir.dt.float32
BF16 = mybir.dt.bfloat16
F32R = mybir.dt.float32r

MM_DTYPE = BF16  # dtype of all TensorE-facing operands (except the tiny
                 # fp32 denominator-broadcast matmul)

_NC_CACHE = {}


def _build_nc(t_len=T, mm_dtype=MM_DTYPE):
    """Build + compile the single-core SPMD program (same NEFF on all cores)."""
    NW = t_len // W  # windows
    SBW = W // 128  # s-chunks per window
    NSB = t_len // 128  # total s-chunks
    DT = mm_dtype

    nc = bacc.Bacc("TRN2", target_bir_lowering=False, debug=False,
                   num_devices=N_CORES)

    xt_d = nc.dram_tensor("xt", [D, t_len], DT, kind="ExternalInput")
    wqk_d = nc.dram_tensor("wqk", [128, KC, 2 * HPC * DH], DT, kind="ExternalInput")
    wv_d = nc.dram_tensor("wv", [128, KC, HPC * DH], DT, kind="ExternalInput")
    wql_d = nc.dram_tensor("wql", [128, HPC // 2, 2 * RANK], DT, kind="ExternalInput")
    wkl_d = nc.dram_tensor("wkl", [128, HPC // 2, 2 * RANK], DT, kind="ExternalInput")
    wo_d = nc.dram_tensor("wo", [128, 2, D], DT, kind="ExternalInput")
    mask_d = nc.dram_tensor("masks", [128, SBW, 2, W], DT, kind="ExternalInput")
    out_d = nc.dram_tensor("out", [t_len, D], F32, kind="ExternalOutput")

    with tile.TileContext(nc) as tc, ExitStack() as ctx:
        const = ctx.enter_context(tc.tile_pool(name="const", bufs=1))
        xt_pool = ctx.enter_context(tc.tile_pool(name="xtp", bufs=2))
        qkt_pool = ctx.enter_context(tc.tile_pool(name="qktp", bufs=2))
        klt_pool = ctx.enter_context(tc.tile_pool(name="kltp", bufs=NW))
        qlt_pool = ctx.enter_context(tc.tile_pool(name="qltp", bufs=2))
        v_pool = ctx.enter_context(tc.tile_pool(name="vp", bufs=NSB))
        p_pool = ctx.enter_context(tc.tile_pool(name="pp", bufs=3))
        y_pool = ctx.enter_context(tc.tile_pool(name="yp", bufs=2))
        yun_pool = ctx.enter_context(tc.tile_pool(name="yunp", bufs=2 * HPC))
        dr_pool = ctx.enter_context(tc.tile_pool(name="drp", bufs=4))
        stage_pool = ctx.enter_context(tc.tile_pool(name="stgp", bufs=2))
        dram_pool = ctx.enter_context(tc.tile_pool(name="dramp", bufs=4,
                                                   space="DRAM"))
        acc = ctx.enter_context(tc.tile_pool(name="accp", bufs=4, space="PSUM"))
        stp = ctx.enter_context(tc.tile_pool(name="stpp", bufs=2, space="PSUM"))

        wqk_sb = const.tile([128, KC, 2 * HPC * DH], DT)
        nc.sync.dma_start(wqk_sb[:], wqk_d.ap()[:, :, :])
        wv_sb = const.tile([128, KC, HPC * DH], DT)
        nc.sync.dma_start(wv_sb[:], wv_d.ap()[:, :, :])
        wql_sb = const.tile([128, HPC // 2, 2 * RANK], DT)
        nc.sync.dma_start(wql_sb[:], wql_d.ap()[:, :, :])
        wkl_sb = const.tile([128, HPC // 2, 2 * RANK], DT)
        nc.sync.dma_start(wkl_sb[:], wkl_d.ap()[:, :, :])
        wo_sb = const.tile([128, 2, D], DT)
        nc.sync.dma_start(wo_sb[:], wo_d.ap()[:, :, :])
        mask_sb = const.tile([128, SBW, 2, W], DT)
        nc.sync.dma_start(mask_sb[:], mask_d.ap()[:, :, :, :])
        ones_sb = const.tile([1, DH], F32)
        nc.vector.memset(ones_sb[:], 1.0)

        xt_dram = xt_d.ap().rearrange("(kc p) t -> p kc t", p=128)

        klt_tiles = [None] * NW
        qlt_tiles = [None] * NW
        v_tiles = [None] * NSB

        # ------------------------------------------------------------------
        # Emission is a "zipper": the attention pair-chain of window w (ACT
        # exp-bound, ~60% PE duty on its own) is interleaved with filler PE
        # work — the QKV/LSR matmul groups of window w+1 and the norm+W_o
        # groups of window w-1 — so the PE instruction stream never starves
        # and the HAM clock gate stays at 8/8.
        # ------------------------------------------------------------------

        def qkv_lsr_closures(w):
            """Filler closures producing qt/kt/v/qlt/klt for window w."""
            xt_w = xt_pool.tile([128, KC, W], DT, tag="xt", name=f"xt_{w}")
            nc.sync.dma_start(xt_w[:], xt_dram[:, :, w * W:(w + 1) * W])
            qt_w = qkt_pool.tile([128, 2, W], DT, tag="qt", name=f"qt_{w}")
            kt_w = qkt_pool.tile([128, 2, W], DT, tag="kt", name=f"kt_{w}")
            qlt_w = qlt_pool.tile([128, W], DT, tag="qlt", name=f"qlt_{w}")
            klt_w = klt_pool.tile([128, W], DT, tag="klt", name=f"klt_{w}")
            qlt_tiles[w] = qlt_w
            klt_tiles[w] = klt_w
            cls = []

            def qk_group(pc, qt_w=qt_w, kt_w=kt_w, xt_w=xt_w):
                ps = acc.tile([128, W], F32, tag="acc")
                for kc in range(KC):
                    nc.tensor.matmul(
                        ps[:],
                        wqk_sb[:, kc, pc * 128:(pc + 1) * 128],
                        xt_w[:, kc, :],
                        start=(kc == 0), stop=(kc == KC - 1),
                    )
                dst = (qt_w if pc < 2 else kt_w)[:, pc % 2, :]
                nc.vector.tensor_copy(dst, ps[:])

            def v_group(tl, xt_w=xt_w, w=w):
                sb = w * SBW + tl
                ps = acc.tile([128, HPC * DH], F32, tag="acc")
                for kc in range(KC):
                    nc.tensor.matmul(
                        ps[:],
                        xt_w[:, kc, tl * 128:(tl + 1) * 128],
                        wv_sb[:, kc, :],
                        start=(kc == 0), stop=(kc == KC - 1),
                    )
                vt = v_pool.tile([128, HPC, DH + 1], DT, tag="v")
                nc.vector.tensor_copy(
                    vt[:, :, 0:DH], ps[:].rearrange("p (h d) -> p h d", h=HPC)
                )
                nc.vector.memset(vt[:, :, DH:DH + 1], 1.0)
                v_tiles[sb] = vt

            def lsr_group(pair, qt_w=qt_w, kt_w=kt_w, qlt_w=qlt_w, klt_w=klt_w):
                ps = acc.tile([64, W], F32, tag="acc")
                nc.tensor.matmul(ps[:], wql_sb[:, pair, :], qt_w[:, pair, :],
                                 start=True, stop=True)
                nc.vector.tensor_copy(qlt_w[pair * 64:(pair + 1) * 64, :], ps[:])
                ps = acc.tile([64, W], F32, tag="acc")
                nc.tensor.matmul(ps[:], wkl_sb[:, pair, :], kt_w[:, pair, :],
                                 start=True, stop=True)
                nc.vector.tensor_copy(klt_w[pair * 64:(pair + 1) * 64, :], ps[:])

            for pc in range(4):
                cls.append(lambda pc=pc: qk_group(pc))
            for tl in range(SBW):
                cls.append(lambda tl=tl: v_group(tl))
            for pair in range(2):
                cls.append(lambda pair=pair: lsr_group(pair))
            return cls

        def norm_wo_closures(w, yun_tiles):
            """Filler closures: normalize window w's y^T, then W_o groups."""
            y_w = y_pool.tile([128, 2, W], DT, tag="y", name=f"y_{w}")
            cls = []

            def norm_head(h, y_w=y_w, yun_tiles=yun_tiles):
                yun = yun_tiles[h]
                rec = dr_pool.tile([1, W], F32, tag="rec")
                nc.vector.reciprocal(rec[:], yun[DH:DH + 1, :])
                # broadcast 1/den across 64 partitions via a DRAM bounce +
                # stride-0-partition DMA instead of a K=1 fp32 matmul: keeps
                # the whole broadcast off the PE stream.
                rec_d = dram_pool.tile([1, W], F32, tag="recd")
                nc.sync.dma_start(rec_d[:], rec[:])
                bc_sb = dr_pool.tile([64, W], F32, tag="bcs")
                nc.sync.dma_start(bc_sb[:], rec_d[:].to_broadcast([64, W]))
                nc.vector.tensor_mul(
                    y_w[(h % 2) * 64:(h % 2) * 64 + 64, h // 2, :],
                    yun[0:DH, :], bc_sb[:],
                )

            def wo_group(tl, y_w=y_w, w=w):
                stg = stage_pool.tile([128, D], F32, tag="stg")
                for nch in range(D // W):
                    ps = acc.tile([128, W], F32, tag="acc")
                    for kc2 in range(2):
                        nc.tensor.matmul(
                            ps[:],
                            y_w[:, kc2, tl * 128:(tl + 1) * 128],
                            wo_sb[:, kc2, nch * W:(nch + 1) * W],
                            start=(kc2 == 0), stop=(kc2 == 1),
                        )
                    nc.vector.tensor_copy(stg[:, nch * W:(nch + 1) * W], ps[:])
                r0 = w * W + tl * 128
                nc.sync.dma_start(out_d.ap()[r0:r0 + 128, :], stg[:])

            for h in range(HPC):
                cls.append(lambda h=h: norm_head(h))
            for tl in range(SBW):
                cls.append(lambda tl=tl: wo_group(tl))
            return cls

        def attention_pairs(w):
            """Attention closures for window w: heads processed in pairs so
            the two scores matmuls land on alternating PE row-strips (they
            execute concurrently) and one exp covers both heads."""
            n_sb = (w + 1) * SBW
            qlt_w = qlt_tiles[w]
            yun_tiles = {}
            state = {}
            cls = []

            def st_exp(hp, sb, qlt_w=qlt_w, w=w):
                sp = stp.tile([128, 2, W], F32, tag="st")
                for j in (0, 1):
                    h = 2 * hp + j
                    nc.tensor.matmul(
                        sp[:, j, :],
                        klt_tiles[sb // SBW][32 * h:32 * h + 32,
                                             (sb % SBW) * 128:(sb % SBW + 1) * 128],
                        qlt_w[32 * h:32 * h + 32, :],
                        start=True, stop=True,
                        tile_position=(32 * h, 0),
                    )
                pt = p_pool.tile([128, 2, W], DT, tag="pt")
                nc.scalar.activation(pt[:], sp[:],
                                     mybir.ActivationFunctionType.Exp)
                if sb // SBW == w:  # diagonal window -> causal mask
                    nc.vector.tensor_mul(pt[:], pt[:],
                                         mask_sb[:, sb % SBW, :, :])
                state[(hp, sb)] = pt

            def pv(hp, sb, n_sb=n_sb):
                if sb == 0:
                    for j in (0, 1):
                        h = 2 * hp + j
                        state[("yps", h)] = acc.tile([DH + 1, W], F32,
                                                     tag="acc", name=f"yps_{h}")
                pt = state.pop((hp, sb))
                for j in (0, 1):
                    h = 2 * hp + j
                    nc.tensor.matmul(
                        state[("yps", h)][:], v_tiles[sb][:, h, :], pt[:, j, :],
                        start=(sb == 0), stop=(sb == n_sb - 1),
                    )
                if sb == n_sb - 1:
                    for j in (0, 1):
                        h = 2 * hp + j
                        yun = yun_pool.tile([DH + 1, W], F32, tag="yun")
                        nc.vector.tensor_copy(yun[:], state[("yps", h)][:])
                        yun_tiles[h] = yun
                        state.pop(("yps", h))

            for hp in range(HPC // 2):
                cls.append(lambda hp=hp: st_exp(hp, 0))
                for sb in range(n_sb):
                    if sb + 1 < n_sb:
                        cls.append(lambda hp=hp, sb=sb: st_exp(hp, sb + 1))
                    cls.append(lambda hp=hp, sb=sb: pv(hp, sb))
            return cls, yun_tiles

        def zip_emit(pairs, fillers):
            """Emit pair closures with filler closures spread between them."""
            nf, np_ = len(fillers), max(1, len(pairs))
            fi = 0
            for k, p in enumerate(pairs):
                p()
                target = (k + 1) * nf // np_
                while fi < target:
                    fillers[fi]()
                    fi += 1
            while fi < nf:
                fillers[fi]()
                fi += 1

        # startup: window 0 inputs
        for c in qkv_lsr_closures(0):
            c()

        pending = None  # (w, yun_tiles) awaiting norm+Wo
        for w in range(NW):
            pairs, yun_tiles = attention_pairs(w)
            fillers = []
            if w + 1 < NW:
                fillers += qkv_lsr_closures(w + 1)
            if pending is not None:
                fillers += norm_wo_closures(*pending)
            zip_emit(pairs, fillers)
            pending = (w, yun_tiles)

        for c in norm_wo_closures(*pending):
            c()

    nc.compile()
    return nc


def _get_nc(t_len=T, mm_dtype=MM_DTYPE):
    key = (t_len, mm_dtype)
    if key not in _NC_CACHE:
        _NC_CACHE[key] = _build_nc(t_len, mm_dtype)
    return _NC_CACHE[key]


def _np_dt(mm_dtype):
    return mybir.dt.np(mm_dtype)


def _build_masks(t_len=T):
    SBW = W // 128
    p = np.arange(128)[:, None]
    c = np.arange(W)[None, :]
    masks = np.zeros((128, SBW, W), np.float32)
    for m in range(SBW):
        masks[:, m, :] = (c - p >= 128 * m).astype(np.float32)
    return masks


def make_in_maps(x, W_qkv, W_q_lsr, W_k_lsr, lsr_core, W_o, t_len=T,
                 mm_dtype=MM_DTYPE):
    x = np.asarray(x, np.float32)
    W_qkv = np.asarray(W_qkv, np.float32)
    W_q_lsr = np.asarray(W_q_lsr, np.float32)
    W_k_lsr = np.asarray(W_k_lsr, np.float32)
    lsr_core = np.asarray(lsr_core, np.float32)
    W_o = np.asarray(W_o, np.float32)
    ndt = _np_dt(mm_dtype)

    d = x.shape[-1]
    masks = np.repeat(_build_masks(t_len)[:, :, None, :], 2, axis=2).astype(ndt)
    scale = lsr_core / np.sqrt(np.float32(RANK))  # [H, R]

    in_maps = []
    for i in range(N_CORES):
        b = i // CPB
        hg = i % CPB
        h0 = hg * HPC
        col0 = h0 * DH
        wq = W_qkv[:, col0:col0 + HPC * DH]
        wk = W_qkv[:, d + col0:d + col0 + HPC * DH]
        wv = W_qkv[:, 2 * d + col0:2 * d + col0 + HPC * DH]
        wqk = np.concatenate([wq, wk], axis=1).reshape(KC, 128, 2 * HPC * DH)
        wqk = wqk.transpose(1, 0, 2)
        wv_p = wv.reshape(KC, 128, HPC * DH).transpose(1, 0, 2)
        wql = np.zeros((HPC // 2, 128, 2 * RANK), np.float32)
        wkl = np.zeros((HPC // 2, 128, 2 * RANK), np.float32)
        for pair in range(HPC // 2):
            for k in range(2):
                h = h0 + pair * 2 + k
                wql[pair, k * DH:(k + 1) * DH, k * RANK:(k + 1) * RANK] = (
                    W_q_lsr[h] * scale[h][None, :]
                )
                wkl[pair, k * DH:(k + 1) * DH, k * RANK:(k + 1) * RANK] = W_k_lsr[h]
        wo = W_o[col0:col0 + HPC * DH, :].reshape(2, 128, d)
        in_maps.append({
            "xt": np.ascontiguousarray(x[b, :t_len].T).astype(ndt),
            "wqk": np.ascontiguousarray(wqk).astype(ndt),
            "wv": np.ascontiguousarray(wv_p).astype(ndt),
            "wql": np.ascontiguousarray(wql.transpose(1, 0, 2)).astype(ndt),
            "wkl": np.ascontiguousarray(wkl.transpose(1, 0, 2)).astype(ndt),
            "wo": np.ascontiguousarray(wo.transpose(1, 0, 2)).astype(ndt),
            "masks": masks,
        })
    return in_maps


def gather_output(results, t_len=T):
    outs = [np.asarray(results[i]["out"], np.float32) for i in range(N_CORES)]
    full = np.stack(
        [sum(outs[b * CPB:(b + 1) * CPB]) for b in range(B)], axis=0
    )
    return full.astype(np.float32)


def run_sharded(inputs, trace=False, t_len=T, mm_dtype=MM_DTYPE):
    nc = _get_nc(t_len, mm_dtype)
    in_maps = make_in_maps(t_len=t_len, mm_dtype=mm_dtype, **inputs)
    res = run_bass_kernel_spmd(nc, in_maps, core_ids=list(range(N_CORES)),
                               trace=trace)
    return gather_output(res.results, t_len), res


def kernel(x, W_qkv, W_q_lsr, W_k_lsr, lsr_core, W_o):
    out, _ = run_sharded(dict(x=x, W_qkv=W_qkv, W_q_lsr=W_q_lsr,
                              W_k_lsr=W_k_lsr, lsr_core=lsr_core, W_o=W_o))
    return out


# revision 15
# speedup vs baseline: 4.1209x; 1.1625x over previous
"""Multi-head self-attention with low-rank bilinear scores (LSR) on 8 TRN2 cores.

Sharding: core i handles batch b = i//4 and 4 heads (i%4)*4..+4.
Per core, everything runs in the "transposed" orientation:
  qkv^T = W^T @ x^T   (contract over D, x pre-transposed on host)
  ql^T / kl^T via block-diagonal packed LSR weights (2 heads per matmul)
  scores^T [s, t] = kl^T-slices @ ql^T  (K = RANK = 32, per-head 32-row strips)
  P = exp(scores^T) * causal_mask;  PV uses V with an appended ones column so
  the softmax denominator falls out of the same matmul (row 64 of y_psum).
  y^T gets normalized by 1/den (broadcast via a K=1 ones matmul), then the
  W_o row-slice projection produces a per-core partial [T, D] output.
Host: per-batch sum over the 4 cores sharing that batch (W_o is row-sharded),
then stack the two batches.
"""

import numpy as np
from contextlib import ExitStack

import concourse.tile as tile
from concourse import bacc, mybir
from concourse.bass_utils import run_bass_kernel_spmd

B, T, D = 2, 2048, 1024
H, RANK = 16, 32
DH = D // H  # 64
N_CORES = 8
HPC = 4  # heads per core
CPB = N_CORES // B  # cores per batch
W = 512  # t-window (= one fp32 PSUM bank)
KC = D // 128  # k-chunks over D
F32 = mybir.dt.float32
BF16 = mybir.dt.bfloat16
F32R = mybir.dt.float32r

MM_DTYPE = BF16  # dtype of all TensorE-facing operands (except the tiny
                 # fp32 denominator-broadcast matmul)

_NC_CACHE = {}


def _build_nc(t_len=T, mm_dtype=MM_DTYPE):
    """Build + compile the single-core SPMD program (same NEFF on all cores)."""
    NW = t_len // W  # windows
    SBW = W // 128  # s-chunks per window
    NSB = t_len // 128  # total s-chunks
    DT = mm_dtype

    nc = bacc.Bacc("TRN2", target_bir_lowering=False, debug=False,
                   num_devices=N_CORES)

    xt_d = nc.dram_tensor("xt", [D, t_len], DT, kind="ExternalInput")
    wqk_d = nc.dram_tensor("wqk", [128, KC, 2 * HPC * DH], DT, kind="ExternalInput")
    wv_d = nc.dram_tensor("wv", [128, KC, HPC * DH], DT, kind="ExternalInput")
    wql_d = nc.dram_tensor("wql", [128, HPC // 2, 2 * RANK], DT, kind="ExternalInput")
    wkl_d = nc.dram_tensor("wkl", [128, HPC // 2, 2 * RANK], DT, kind="ExternalInput")
    wo_d = nc.dram_tensor("wo", [128, 2, D], DT, kind="ExternalInput")
    mask_d = nc.dram_tensor("masks", [128, SBW, 2, W], DT, kind="ExternalInput")
    out_d = nc.dram_tensor("out", [t_len, D], F32, kind="ExternalOutput")

    with tile.TileContext(nc) as tc, ExitStack() as ctx:
        const = ctx.enter_context(tc.tile_pool(name="const", bufs=1))
        xt_pool = ctx.enter_context(tc.tile_pool(name="xtp", bufs=2))
        qkt_pool = ctx.enter_context(tc.tile_pool(name="qktp", bufs=2))
        klt_pool = ctx.enter_context(tc.tile_pool(name="kltp", bufs=NW))
        qlt_pool = ctx.enter_context(tc.tile_pool(name="qltp", bufs=2))
        v_pool = ctx.enter_context(tc.tile_pool(name="vp", bufs=NSB))
        p_pool = ctx.enter_context(tc.tile_pool(name="pp", bufs=3))
        y_pool = ctx.enter_context(tc.tile_pool(name="yp", bufs=2))
        yun_pool = ctx.enter_context(tc.tile_pool(name="yunp", bufs=2 * HPC))
        dr_pool = ctx.enter_context(tc.tile_pool(name="drp", bufs=4))
        stage_pool = ctx.enter_context(tc.tile_pool(name="stgp", bufs=2))
        dram_pool = ctx.enter_context(tc.tile_pool(name="dramp", bufs=4,
                                                   space="DRAM"))
        acc = ctx.enter_context(tc.tile_pool(name="accp", bufs=4, space="PSUM"))
        stp = ctx.enter_context(tc.tile_pool(name="stpp", bufs=2, space="PSUM"))

        wqk_sb = const.tile([128, KC, 2 * HPC * DH], DT)
        nc.sync.dma_start(wqk_sb[:], wqk_d.ap()[:, :, :])
        wv_sb = const.tile([128, KC, HPC * DH], DT)
        nc.sync.dma_start(wv_sb[:], wv_d.ap()[:, :, :])
        wql_sb = const.tile([128, HPC // 2, 2 * RANK], DT)
        nc.sync.dma_start(wql_sb[:], wql_d.ap()[:, :, :])
        wkl_sb = const.tile([128, HPC // 2, 2 * RANK], DT)
        nc.sync.dma_start(wkl_sb[:], wkl_d.ap()[:, :, :])
        wo_sb = const.tile([128, 2, D], DT)
        nc.sync.dma_start(wo_sb[:], wo_d.ap()[:, :, :])
        mask_sb = const.tile([128, SBW, 2, W], DT)
        nc.sync.dma_start(mask_sb[:], mask_d.ap()[:, :, :, :])
        ones_sb = const.tile([1, DH], F32)
        nc.vector.memset(ones_sb[:], 1.0)

        xt_dram = xt_d.ap().rearrange("(kc p) t -> p kc t", p=128)

        klt_tiles = [None] * NW
        qlt_tiles = [None] * NW
        v_tiles = [None] * NSB

        # ------------------------------------------------------------------
        # Emission is a "zipper": the attention pair-chain of window w (ACT
        # exp-bound, ~60% PE duty on its own) is interleaved with filler PE
        # work — the QKV/LSR matmul groups of window w+1 and the norm+W_o
        # groups of window w-1 — so the PE instruction stream never starves
        # and the HAM clock gate stays at 8/8.
        # ------------------------------------------------------------------

        def qkv_lsr_closures(w):
            """Filler closures producing qt/kt/v/qlt/klt for window w."""
            xt_w = xt_pool.tile([128, KC, W], DT, tag="xt", name=f"xt_{w}")
            nc.sync.dma_start(xt_w[:], xt_dram[:, :, w * W:(w + 1) * W])
            qt_w = qkt_pool.tile([128, 2, W], DT, tag="qt", name=f"qt_{w}")
            kt_w = qkt_pool.tile([128, 2, W], DT, tag="kt", name=f"kt_{w}")
            qlt_w = qlt_pool.tile([128, W], DT, tag="qlt", name=f"qlt_{w}")
            klt_w = klt_pool.tile([128, W], DT, tag="klt", name=f"klt_{w}")
            qlt_tiles[w] = qlt_w
            klt_tiles[w] = klt_w
            cls = []

            def qk_group(pc, qt_w=qt_w, kt_w=kt_w, xt_w=xt_w):
                ps = acc.tile([128, W], F32, tag="acc")
                for kc in range(KC):
                    nc.tensor.matmul(
                        ps[:],
                        wqk_sb[:, kc, pc * 128:(pc + 1) * 128],
                        xt_w[:, kc, :],
                        start=(kc == 0), stop=(kc == KC - 1),
                    )
                dst = (qt_w if pc < 2 else kt_w)[:, pc % 2, :]
                nc.vector.tensor_copy(dst, ps[:])

            def v_group(tl, xt_w=xt_w, w=w):
                sb = w * SBW + tl
                ps = acc.tile([128, HPC * DH], F32, tag="acc")
                for kc in range(KC):
                    nc.tensor.matmul(
                        ps[:],
                        xt_w[:, kc, tl * 128:(tl + 1) * 128],
                        wv_sb[:, kc, :],
                        start=(kc == 0), stop=(kc == KC - 1),
                    )
                vt = v_pool.tile([128, HPC, DH + 1], DT, tag="v")
                nc.vector.tensor_copy(
                    vt[:, :, 0:DH], ps[:].rearrange("p (h d) -> p h d", h=HPC)
                )
                nc.vector.memset(vt[:, :, DH:DH + 1], 1.0)
                v_tiles[sb] = vt

            def lsr_group(pair, qt_w=qt_w, kt_w=kt_w, qlt_w=qlt_w, klt_w=klt_w):
                ps = acc.tile([64, W], F32, tag="acc")
                nc.tensor.matmul(ps[:], wql_sb[:, pair, :], qt_w[:, pair, :],
                                 start=True, stop=True)
                nc.vector.tensor_copy(qlt_w[pair * 64:(pair + 1) * 64, :], ps[:])
                ps = acc.tile([64, W], F32, tag="acc")
                nc.tensor.matmul(ps[:], wkl_sb[:, pair, :], kt_w[:, pair, :],
                                 start=True, stop=True)
                nc.vector.tensor_copy(klt_w[pair * 64:(pair + 1) * 64, :], ps[:])

            for pc in range(4):
                cls.append(lambda pc=pc: qk_group(pc))
            for tl in range(SBW):
                cls.append(lambda tl=tl: v_group(tl))
            for pair in range(2):
                cls.append(lambda pair=pair: lsr_group(pair))
            return cls

        def norm_wo_closures(w, yun_tiles):
            """Filler closures: normalize window w's y^T, then W_o groups."""
            y_w = y_pool.tile([128, 2, W], DT, tag="y", name=f"y_{w}")
            cls = []
            shared = {}

            def recip_all(yun_tiles=yun_tiles):
                # gather the 4 heads' denominators (partition moves are fine
                # for DMA), take ONE reciprocal for all heads (cost is per
                # lane-element, so [4, W] == [1, W]), bounce to DRAM for the
                # stride-0-partition broadcast reads.
                den4 = dr_pool.tile([HPC, W], F32, tag="den4")
                for h in range(HPC):
                    nc.sync.dma_start(den4[h:h + 1, :],
                                      yun_tiles[h][DH:DH + 1, :])
                rec4 = dr_pool.tile([HPC, W], F32, tag="rec4")
                nc.vector.reciprocal(rec4[:], den4[:])
                rec4_d = dram_pool.tile([HPC, W], F32, tag="recd")
                nc.sync.dma_start(rec4_d[:], rec4[:])
                shared["rec4_d"] = rec4_d

            def norm_head(h, y_w=y_w, yun_tiles=yun_tiles):
                yun = yun_tiles[h]
                bc_sb = dr_pool.tile([64, W], F32, tag="bcs")
                nc.sync.dma_start(
                    bc_sb[:], shared["rec4_d"][h:h + 1, :].to_broadcast([64, W])
                )
                nc.vector.tensor_mul(
                    y_w[(h % 2) * 64:(h % 2) * 64 + 64, h // 2, :],
                    yun[0:DH, :], bc_sb[:],
                )

            def wo_group(tl, y_w=y_w, w=w):
                stg = stage_pool.tile([128, D], F32, tag="stg")
                for nch in range(D // W):
                    ps = acc.tile([128, W], F32, tag="acc")
                    for kc2 in range(2):
                        nc.tensor.matmul(
                            ps[:],
                            y_w[:, kc2, tl * 128:(tl + 1) * 128],
                            wo_sb[:, kc2, nch * W:(nch + 1) * W],
                            start=(kc2 == 0), stop=(kc2 == 1),
                        )
                    nc.vector.tensor_copy(stg[:, nch * W:(nch + 1) * W], ps[:])
                r0 = w * W + tl * 128
                nc.sync.dma_start(out_d.ap()[r0:r0 + 128, :], stg[:])

            cls.append(recip_all)
            for h in range(HPC):
                cls.append(lambda h=h: norm_head(h))
            for tl in range(SBW):
                cls.append(lambda tl=tl: wo_group(tl))
            return cls

        def attention_pairs(w):
            """Attention closures for window w: heads processed in pairs so
            the two scores matmuls land on alternating PE row-strips (they
            execute concurrently) and one exp covers both heads."""
            n_sb = (w + 1) * SBW
            qlt_w = qlt_tiles[w]
            yun_tiles = {}
            state = {}
            cls = []

            def st_exp(hp, sb, qlt_w=qlt_w, w=w):
                sp = stp.tile([128, 2, W], F32, tag="st")
                for j in (0, 1):
                    h = 2 * hp + j
                    nc.tensor.matmul(
                        sp[:, j, :],
                        klt_tiles[sb // SBW][32 * h:32 * h + 32,
                                             (sb % SBW) * 128:(sb % SBW + 1) * 128],
                        qlt_w[32 * h:32 * h + 32, :],
                        start=True, stop=True,
                        tile_position=(32 * h, 0),
                    )
                pt = p_pool.tile([128, 2, W], DT, tag="pt")
                nc.scalar.activation(pt[:], sp[:],
                                     mybir.ActivationFunctionType.Exp)
                if sb // SBW == w:  # diagonal window -> causal mask
                    nc.vector.tensor_mul(pt[:], pt[:],
                                         mask_sb[:, sb % SBW, :, :])
                state[(hp, sb)] = pt

            def pv(hp, sb, n_sb=n_sb):
                if sb == 0:
                    for j in (0, 1):
                        h = 2 * hp + j
                        state[("yps", h)] = acc.tile([DH + 1, W], F32,
                                                     tag="acc", name=f"yps_{h}")
                pt = state.pop((hp, sb))
                for j in (0, 1):
                    h = 2 * hp + j
                    nc.tensor.matmul(
                        state[("yps", h)][:], v_tiles[sb][:, h, :], pt[:, j, :],
                        start=(sb == 0), stop=(sb == n_sb - 1),
                    )
                if sb == n_sb - 1:
                    for j in (0, 1):
                        h = 2 * hp + j
                        yun = yun_pool.tile([DH + 1, W], F32, tag="yun")
                        nc.vector.tensor_copy(yun[:], state[("yps", h)][:])
                        yun_tiles[h] = yun
                        state.pop(("yps", h))

            for hp in range(HPC // 2):
                cls.append(lambda hp=hp: st_exp(hp, 0))
                for sb in range(n_sb):
                    if sb + 1 < n_sb:
                        cls.append(lambda hp=hp, sb=sb: st_exp(hp, sb + 1))
                    cls.append(lambda hp=hp, sb=sb: pv(hp, sb))
            return cls, yun_tiles

        def zip_emit(pairs, fillers):
            """Emit pair closures with filler closures spread between them."""
            nf, np_ = len(fillers), max(1, len(pairs))
            fi = 0
            for k, p in enumerate(pairs):
                p()
                target = (k + 1) * nf // np_
                while fi < target:
                    fillers[fi]()
                    fi += 1
            while fi < nf:
                fillers[fi]()
                fi += 1

        # startup: window 0 inputs
        for c in qkv_lsr_closures(0):
            c()

        pending = None  # (w, yun_tiles) awaiting norm+Wo
        for w in range(NW):
            pairs, yun_tiles = attention_pairs(w)
            fillers = []
            if w + 1 < NW:
                fillers += qkv_lsr_closures(w + 1)
            if pending is not None:
                fillers += norm_wo_closures(*pending)
            zip_emit(pairs, fillers)
            pending = (w, yun_tiles)

        for c in norm_wo_closures(*pending):
            c()

    nc.compile()
    return nc


def _get_nc(t_len=T, mm_dtype=MM_DTYPE):
    key = (t_len, mm_dtype)
    if key not in _NC_CACHE:
        _NC_CACHE[key] = _build_nc(t_len, mm_dtype)
    return _NC_CACHE[key]


def _np_dt(mm_dtype):
    return mybir.dt.np(mm_dtype)


def _build_masks(t_len=T):
    SBW = W // 128
    p = np.arange(128)[:, None]
    c = np.arange(W)[None, :]
    masks = np.zeros((128, SBW, W), np.float32)
    for m in range(SBW):
        masks[:, m, :] = (c - p >= 128 * m).astype(np.float32)
    return masks


def make_in_maps(x, W_qkv, W_q_lsr, W_k_lsr, lsr_core, W_o, t_len=T,
                 mm_dtype=MM_DTYPE):
    x = np.asarray(x, np.float32)
    W_qkv = np.asarray(W_qkv, np.float32)
    W_q_lsr = np.asarray(W_q_lsr, np.float32)
    W_k_lsr = np.asarray(W_k_lsr, np.float32)
    lsr_core = np.asarray(lsr_core, np.float32)
    W_o = np.asarray(W_o, np.float32)
    ndt = _np_dt(mm_dtype)

    d = x.shape[-1]
    masks = np.repeat(_build_masks(t_len)[:, :, None, :], 2, axis=2).astype(ndt)
    scale = lsr_core / np.sqrt(np.float32(RANK))  # [H, R]

    in_maps = []
    for i in range(N_CORES):
        b = i // CPB
        hg = i % CPB
        h0 = hg * HPC
        col0 = h0 * DH
        wq = W_qkv[:, col0:col0 + HPC * DH]
        wk = W_qkv[:, d + col0:d + col0 + HPC * DH]
        wv = W_qkv[:, 2 * d + col0:2 * d + col0 + HPC * DH]
        wqk = np.concatenate([wq, wk], axis=1).reshape(KC, 128, 2 * HPC * DH)
        wqk = wqk.transpose(1, 0, 2)
        wv_p = wv.reshape(KC, 128, HPC * DH).transpose(1, 0, 2)
        wql = np.zeros((HPC // 2, 128, 2 * RANK), np.float32)
        wkl = np.zeros((HPC // 2, 128, 2 * RANK), np.float32)
        for pair in range(HPC // 2):
            for k in range(2):
                h = h0 + pair * 2 + k
                wql[pair, k * DH:(k + 1) * DH, k * RANK:(k + 1) * RANK] = (
                    W_q_lsr[h] * scale[h][None, :]
                )
                wkl[pair, k * DH:(k + 1) * DH, k * RANK:(k + 1) * RANK] = W_k_lsr[h]
        wo = W_o[col0:col0 + HPC * DH, :].reshape(2, 128, d)
        in_maps.append({
            "xt": np.ascontiguousarray(x[b, :t_len].T).astype(ndt),
            "wqk": np.ascontiguousarray(wqk).astype(ndt),
            "wv": np.ascontiguousarray(wv_p).astype(ndt),
            "wql": np.ascontiguousarray(wql.transpose(1, 0, 2)).astype(ndt),
            "wkl": np.ascontiguousarray(wkl.transpose(1, 0, 2)).astype(ndt),
            "wo": np.ascontiguousarray(wo.transpose(1, 0, 2)).astype(ndt),
            "masks": masks,
        })
    return in_maps


def gather_output(results, t_len=T):
    outs = [np.asarray(results[i]["out"], np.float32) for i in range(N_CORES)]
    full = np.stack(
        [sum(outs[b * CPB:(b + 1) * CPB]) for b in range(B)], axis=0
    )
    return full.astype(np.float32)


def run_sharded(inputs, trace=False, t_len=T, mm_dtype=MM_DTYPE):
    nc = _get_nc(t_len, mm_dtype)
    in_maps = make_in_maps(t_len=t_len, mm_dtype=mm_dtype, **inputs)
    res = run_bass_kernel_spmd(nc, in_maps, core_ids=list(range(N_CORES)),
                               trace=trace)
    return gather_output(res.results, t_len), res


def kernel(x, W_qkv, W_q_lsr, W_k_lsr, lsr_core, W_o):
    out, _ = run_sharded(dict(x=x, W_qkv=W_qkv, W_q_lsr=W_q_lsr,
                              W_k_lsr=W_k_lsr, lsr_core=lsr_core, W_o=W_o))
    return out
